# revision 9
# baseline (speedup 1.0000x reference)
"""Trainium2 Bass kernel for nn_MoEMLABlock (MoE + multi-level attention block).

Strategy (8 NeuronCores, full inputs in / full output out):
  Launch A (head-parallel attention): 64 attention instances (level l,
    batch b, head h) are split as: core c -> batch b=c//4, level l=(c%4)//2,
    head block hb=(c%4)%2 (8 heads / 512 feature cols).  Each projection is
    computed exactly once across cores (no K/V recompute).  LayerNorm 1,
    the 1/sqrt(DH) scale and the level-softmax weight are folded into the
    projection weights on the host; V carries no bias (its effect, plus bo,
    is the constant `boc` added on the host).  Device math is bf16 inputs
    with fp32 PSUM accumulation.  Each core emits its partial attention
    output [S, H] token-major; host sums partials + residual.
  Host: exact fp32 attention recompute (BLAS) for ROUTING ONLY -- router
    logits, top-2 and gates are bit-robust against device rounding (the
    tightest p2/p3 logit gap in this data is 3e-5, so routing must not
    depend on device numerics).  Also LN2 + expert input prep.
  Launch B (expert-parallel FFN): core e runs expert e over its routed
    tokens with error-compensated fp8 (e4m3) DoubleRow matmuls:
    x@W1 ~= xm@W1m + xr@W1m + xm@W1r where *m/*r are fp8 main/residual
    parts (measured output error 5e-4 rel, 4x PE throughput per term).
    Gates are applied on device via the activation scale; the b2 bias is
    applied on the host (gates @ b2).
  Host: scatter-add combine + residuals.
"""

import numpy as np

H = 1024
NH = 16
DH = 64
L = 2
E = 8
FF = 4096
B = 2
S = 1024
EPS = 1e-5
P = 128
NCORES = 8
KO = H // P            # 8 contraction chunks over H
TT = S // P            # 8 token tiles
FB = 512               # feature block per core (8 heads)
MF = FF // P           # 32
TOPK = 2

SX = 16.0              # fp8 scale for expert input x
SW = 64.0              # fp8 scale for expert weights

_CACHE = {}
_PERF = {}


def _build_attn():
    """Launch A: one (batch, level, 8-head block) per core."""
    import concourse.bacc as bacc
    import concourse.mybir as mybir
    import concourse.tile as tile

    F32, F32R, BF16 = mybir.dt.float32, mybir.dt.float32r, mybir.dt.bfloat16
    AF = mybir.ActivationFunctionType

    nc = bacc.Bacc()
    xnT_h = nc.dram_tensor("xnT", [H, S], BF16, kind="ExternalInput")   # LN1(x_b)^T
    wq_h = nc.dram_tensor("wq", [H, FB], BF16, kind="ExternalInput")
    wk_h = nc.dram_tensor("wk", [H, FB], BF16, kind="ExternalInput")
    wv_h = nc.dram_tensor("wv", [H, FB], BF16, kind="ExternalInput")
    wo_h = nc.dram_tensor("wo", [FB, H], BF16, kind="ExternalInput")
    bq_h = nc.dram_tensor("bqc", [P, 4], F32, kind="ExternalInput")
    bk_h = nc.dram_tensor("bkc", [P, 4], F32, kind="ExternalInput")
    mb_h = nc.dram_tensor("mb", [P, TT], F32, kind="ExternalInput")     # key mask bias
    po_h = nc.dram_tensor("po", [S, H], F32, kind="ExternalOutput")     # partial attn out

    with tile.TileContext(nc) as tc:
        with tc.tile_pool(name="consts", bufs=1) as consts, \
             tc.tile_pool(name="big", bufs=1) as big, \
             tc.tile_pool(name="work", bufs=2) as work, \
             tc.tile_pool(name="ps_mm", bufs=2, space="PSUM") as ps_mm, \
             tc.tile_pool(name="ps_sc", bufs=2, space="PSUM") as ps_sc, \
             tc.tile_pool(name="ps_cx", bufs=2, space="PSUM") as ps_cx:

            ones_f = consts.tile([P, 1], F32)
            nc.vector.memset(ones_f[:], 1.0)
            ones_row = consts.tile([1, P], F32R)
            nc.vector.tensor_copy(ones_row[:], ones_f[:1, :].to_broadcast((1, P)))

            bq_t = consts.tile([P, 4], F32)
            nc.sync.dma_start(bq_t[:], bq_h[:])
            bk_t = consts.tile([P, 4], F32)
            nc.sync.dma_start(bk_t[:], bk_h[:])
            mb_t = consts.tile([P, TT], F32)
            nc.sync.dma_start(mb_t[:], mb_h[:])

            # DMA order = first-use order: V needs wv + xn(lo) first.
            wv_t = big.tile([P, KO, FB], BF16)
            nc.sync.dma_start(wv_t[:], wv_h[:].rearrange("(ko p) f -> p ko f", p=P))
            xn_a = big.tile([P, KO // 2, S], BF16)
            nc.sync.dma_start(
                xn_a[:], xnT_h[:H // 2].rearrange("(ko p) t -> p ko t", p=P))
            xn_b = big.tile([P, KO // 2, S], BF16)
            nc.sync.dma_start(
                xn_b[:], xnT_h[H // 2:].rearrange("(ko p) t -> p ko t", p=P))
            wq_t = big.tile([P, KO, FB], BF16)
            nc.sync.dma_start(wq_t[:], wq_h[:].rearrange("(ko p) f -> p ko f", p=P))
            wk_t = big.tile([P, KO, FB], BF16)
            nc.sync.dma_start(wk_t[:], wk_h[:].rearrange("(ko p) f -> p ko f", p=P))
            wo_t = big.tile([P, 4, H], BF16)
            nc.sync.dma_start(wo_t[:], wo_h[:].rearrange("(fc p) h -> p fc h", p=P))

            def xn(kc):
                return (xn_a if kc < KO // 2 else xn_b)[:, kc % (KO // 2), :]

            # ---- V token-major, with ones column (row 64) for sumexp ----
            v_t = big.tile([P, TT, 8, DH + 1], BF16)
            nc.vector.memset(v_t[:, :, :, DH:DH + 1], 1.0)
            for tt in range(TT):
                vps = ps_mm.tile([P, FB], F32, tag="mm")
                for kc in range(KO):
                    nc.tensor.matmul(
                        vps[:], xn(kc)[:, tt * P:(tt + 1) * P], wv_t[:, kc, :],
                        start=(kc == 0), stop=(kc == KO - 1),
                    )
                nc.vector.tensor_copy(
                    v_t[:, tt, :, 0:DH],
                    vps[:].rearrange("p (h d) -> p h d", d=DH),
                )

            # ---- Q, K feature-major (f = ft*128 + p), bias via DVE ----
            q_t = big.tile([P, 4, S], BF16)
            k_t = big.tile([P, 4, S], BF16)
            for dst, wsrc, bsrc in ((q_t, wq_t, bq_t), (k_t, wk_t, bk_t)):
                for ft in range(4):
                    for qc in range(2):
                        pps = ps_mm.tile([P, FB], F32, tag="mm")
                        for kc in range(KO):
                            nc.tensor.matmul(
                                pps[:],
                                wsrc[:, kc, ft * P:(ft + 1) * P],
                                xn(kc)[:, qc * FB:(qc + 1) * FB],
                                start=(kc == 0), stop=(kc == KO - 1),
                            )
                        nc.vector.tensor_scalar_add(
                            dst[:, ft, qc * FB:(qc + 1) * FB], pps[:],
                            bsrc[:, ft:ft + 1],
                        )

            # ---- per-head: scores -> exp -> ctx (+sumexp) -> normalize ----
            ctx = big.tile([P, 4, S], BF16)
            for h in range(8):
                ft, pb = h // 2, (h % 2) * DH
                p_sb = work.tile([P, TT, S], BF16, tag="p_sb")
                for tt in range(TT):
                    sps = ps_sc.tile([P, S], F32, tag="sc")
                    for qc in range(2):
                        nc.tensor.matmul(
                            sps[:, qc * FB:(qc + 1) * FB],
                            k_t[pb:pb + DH, ft, tt * P:(tt + 1) * P],
                            q_t[pb:pb + DH, ft, qc * FB:(qc + 1) * FB],
                            start=True, stop=True,
                        )
                    nc.scalar.activation(
                        p_sb[:, tt, :], sps[:],
                        AF.Exp, bias=mb_t[:, tt:tt + 1],
                    )
                for qc in range(2):
                    cxps = ps_cx.tile([DH + 1, FB], F32, tag="cx")
                    for tt in range(TT):
                        nc.tensor.matmul(
                            cxps[:],
                            v_t[:, tt, h, 0:DH + 1],
                            p_sb[:, tt, qc * FB:(qc + 1) * FB],
                            start=(tt == 0), stop=(tt == TT - 1),
                        )
                    r = work.tile([1, FB], F32R, tag="r")
                    with nc.allow_low_precision(reason="softmax recip feeds broadcast matmul"):
                        nc.vector.reciprocal(r[:], cxps[DH:DH + 1, :])
                    rbps = ps_mm.tile([P, FB], F32, tag="mm")
                    nc.tensor.matmul(rbps[:DH, :], ones_row[:, :DH], r[:], start=True, stop=True)
                    rb_sb = work.tile([DH, FB], F32, tag="rb_sb")
                    nc.vector.tensor_copy(rb_sb[:], rbps[:DH, :])
                    nc.vector.tensor_mul(
                        ctx[pb:pb + DH, ft, qc * FB:(qc + 1) * FB],
                        cxps[0:DH, :], rb_sb[:],
                    )

            # ---- O-projection straight to token-major, DMA out ----
            po_t = big.tile([P, TT, H], F32)
            for tt in range(TT):
                for hc in range(2):
                    ops = ps_mm.tile([P, FB], F32, tag="mm")
                    for fc in range(4):
                        nc.tensor.matmul(
                            ops[:],
                            ctx[:, fc, tt * P:(tt + 1) * P],
                            wo_t[:, fc, hc * FB:(hc + 1) * FB],
                            start=(fc == 0), stop=(fc == 3),
                        )
                    nc.vector.tensor_copy(po_t[:, tt, hc * FB:(hc + 1) * FB], ops[:])
                nc.sync.dma_start(
                    po_h[:].rearrange("(tt p) h -> p tt h", p=P)[:, tt, :],
                    po_t[:, tt, :],
                )

    nc.finalize()
    return nc


def _build_expert(C):
    """Launch B: one expert FFN over C routed tokens, compensated fp8."""
    import concourse.bacc as bacc
    import concourse.mybir as mybir
    import concourse.tile as tile

    F32, F8 = mybir.dt.float32, mybir.dt.float8e4
    AF = mybir.ActivationFunctionType
    DR = mybir.MatmulPerfMode.DoubleRow

    assert C % P == 0 and C >= 512, C
    NCH = max(1, (C + 511) // 512)
    assert C % NCH == 0, (C, NCH)
    CN = C // NCH
    assert 256 <= CN <= 512, CN
    CT = C // P

    nc = bacc.Bacc()
    xm_h = nc.dram_tensor("xm", [H, C], F8, kind="ExternalInput")   # SX*x main
    xr_h = nc.dram_tensor("xr", [H, C], F8, kind="ExternalInput")   # SX*x residual
    w1m_h = nc.dram_tensor("w1m", [H, FF], F8, kind="ExternalInput")
    w1r_h = nc.dram_tensor("w1r", [H, FF], F8, kind="ExternalInput")
    w2m_h = nc.dram_tensor("w2m", [FF, H], F8, kind="ExternalInput")
    w2r_h = nc.dram_tensor("w2r", [FF, H], F8, kind="ExternalInput")
    b1_h = nc.dram_tensor("b1c", [P, MF], F32, kind="ExternalInput")
    gs_h = nc.dram_tensor("gsc", [P, CT], F32, kind="ExternalInput")  # gate/SW per token
    y_h = nc.dram_tensor("y", [C, H], F32, kind="ExternalOutput")

    with tile.TileContext(nc) as tc:
        with tc.tile_pool(name="consts", bufs=1) as consts, \
             tc.tile_pool(name="big", bufs=1) as big, \
             tc.tile_pool(name="w1s", bufs=2) as w1s, \
             tc.tile_pool(name="ht", bufs=2) as htp, \
             tc.tile_pool(name="ps_mm", bufs=3, space="PSUM") as ps_mm:

            b1_t = consts.tile([P, MF], F32)
            nc.sync.dma_start(b1_t[:], b1_h[:])
            gs_t = consts.tile([P, CT], F32)
            nc.sync.dma_start(gs_t[:], gs_h[:])

            xm_t = big.tile([P, KO, C], F8)
            nc.sync.dma_start(xm_t[:], xm_h[:].rearrange("(ko p) c -> p ko c", p=P))
            xr_t = big.tile([P, KO, C], F8)
            nc.sync.dma_start(xr_t[:], xr_h[:].rearrange("(ko p) c -> p ko c", p=P))

            hm_t = big.tile([P, MF, C], F8)
            hr_t = big.tile([P, MF, C], F8)

            # ---- W1 pass: h = gelu((xm+xr)@(w1m+w1r)/(SX*SW) + b1) ----
            for mfc in range(FF // FB):            # 8 chunks of 512 cols
                w1m_c = w1s.tile([P, KO, FB], F8, tag="w1m")
                nc.sync.dma_start(
                    w1m_c[:],
                    w1m_h[:, mfc * FB:(mfc + 1) * FB].rearrange("(ko p) f -> p ko f", p=P),
                )
                w1r_c = w1s.tile([P, KO, FB], F8, tag="w1r")
                nc.sync.dma_start(
                    w1r_c[:],
                    w1r_h[:, mfc * FB:(mfc + 1) * FB].rearrange("(ko p) f -> p ko f", p=P),
                )
                for mf in range(4):
                    mfg = mfc * 4 + mf
                    for nch in range(NCH):
                        cs = slice(nch * CN, (nch + 1) * CN)
                        hps = ps_mm.tile([P, 512], F32, tag="mm")
                        terms = [(w1m_c, xm_t), (w1m_c, xr_t), (w1r_c, xm_t)]
                        nterm = len(terms)
                        for ti, (wt, xt) in enumerate(terms):
                            for kc2 in range(KO // 2):
                                nc.tensor.matmul(
                                    hps[:, :CN],
                                    wt[:, 2 * kc2:2 * kc2 + 2, mf * P:(mf + 1) * P],
                                    xt[:, 2 * kc2:2 * kc2 + 2, cs],
                                    start=(ti == 0 and kc2 == 0),
                                    stop=(ti == nterm - 1 and kc2 == KO // 2 - 1),
                                    perf_mode=DR,
                                )
                        ht = htp.tile([P, 512], F32, tag="ht")
                        nc.scalar.activation(
                            ht[:, :CN], hps[:, :CN], AF.Gelu_apprx_tanh,
                            bias=b1_t[:, mfg:mfg + 1], scale=1.0 / (SX * SW),
                        )
                        # split the fp8 quantize-copy across Act/DVE
                        if mfg % 2 == 0:
                            nc.scalar.activation(hm_t[:, mfg, cs], ht[:, :CN], AF.Copy)
                        else:
                            nc.vector.tensor_copy(hm_t[:, mfg, cs], ht[:, :CN])
                        nc.vector.tensor_sub(hr_t[:, mfg, cs], ht[:, :CN], hm_t[:, mfg, cs])

            # ---- W2 pass: y = (h@(w2m+w2r))/SW * gate, token-major ----
            w2m_t = big.tile([P, MF, H], F8)
            nc.sync.dma_start(w2m_t[:], w2m_h[:].rearrange("(fc p) h -> p fc h", p=P))
            w2r_t = big.tile([P, MF, H], F8)
            nc.sync.dma_start(w2r_t[:], w2r_h[:].rearrange("(fc p) h -> p fc h", p=P))
            y_t = big.tile([P, CT, H], F32)
            for ct in range(CT):
                for hc in range(2):
                    yps = ps_mm.tile([P, 512], F32, tag="mm")
                    terms = [(hm_t, w2m_t), (hr_t, w2m_t), (hm_t, w2r_t)]
                    nterm = len(terms)
                    for ti, (ht_, wt) in enumerate(terms):
                        for fc2 in range(MF // 2):
                            nc.tensor.matmul(
                                yps[:],
                                ht_[:, 2 * fc2:2 * fc2 + 2, ct * P:(ct + 1) * P],
                                wt[:, 2 * fc2:2 * fc2 + 2, hc * FB:(hc + 1) * FB],
                                start=(ti == 0 and fc2 == 0),
                                stop=(ti == nterm - 1 and fc2 == MF // 2 - 1),
                                perf_mode=DR,
                            )
                    nc.scalar.activation(
                        y_t[:, ct, hc * FB:(hc + 1) * FB], yps[:],
                        AF.Copy, scale=gs_t[:, ct:ct + 1],
                    )
                nc.sync.dma_start(
                    y_h[:].rearrange("(ct p) h -> p ct h", p=P)[:, ct, :],
                    y_t[:, ct, :],
                )

    nc.finalize()
    return nc


def _get_attn():
    if "attn" not in _CACHE:
        _CACHE["attn"] = _build_attn()
    return _CACHE["attn"]


def _get_expert(C):
    key = ("exp", C)
    if key not in _CACHE:
        _CACHE[key] = _build_expert(C)
    return _CACHE[key]


def _ln(x):
    m = x.mean(-1, keepdims=True)
    v = x.var(-1, keepdims=True)
    return (x - m) / np.sqrt(v + EPS)


def _q8(a, s):
    """e4m3 quantize a*s, return (main, residual) as raw scaled fp8 arrays."""
    import ml_dtypes
    e4 = ml_dtypes.float8_e4m3
    m = (a * s).astype(np.float32).astype(e4)
    r = ((a * s).astype(np.float32) - m.astype(np.float32)).astype(e4)
    return m, r


def _colt(vec):
    v32 = np.ascontiguousarray(np.asarray(vec, dtype=np.float32))
    return np.ascontiguousarray(v32.reshape(-1, P).T)


def kernel(**inputs):
    import ml_dtypes
    from concourse.bass_utils import run_bass_kernel_spmd

    bf16 = ml_dtypes.bfloat16
    f = lambda k: np.asarray(inputs[k], dtype=np.float32)
    x = f("hidden_states")                        # [B, S, H]
    mask = np.asarray(inputs["attention_mask"])   # [B, S] int32
    ln1_g, ln1_b = f("ln1_g").astype(np.float64), f("ln1_b").astype(np.float64)
    ln2_g, ln2_b = f("ln2_g").astype(np.float64), f("ln2_b").astype(np.float64)
    Wq, Wk, Wv, Wo = (f(k).astype(np.float64) for k in ("Wq", "Wk", "Wv", "Wo"))
    bq, bk, bv, bo = (f(k).astype(np.float64) for k in ("bq", "bk", "bv", "bo"))
    level_logits = f("level_logits").astype(np.float64)
    Wr, br = f("Wr").astype(np.float64), f("br").astype(np.float64)
    W1, b1 = f("W1").astype(np.float64), f("b1").astype(np.float64)
    W2, b2 = f("W2").astype(np.float64), f("b2").astype(np.float64)

    # ---- host folding (as in reference, with LN1 gamma/beta absorbed) ----
    scale = 1.0 / np.sqrt(DH)
    wq_eff = (ln1_g[None, :, None] * Wq) * scale               # [L,H,H]
    bq_eff = (bq + ln1_b @ Wq) * scale                         # [L,H]
    wk_eff = ln1_g[None, :, None] * Wk
    bk_eff = bk + ln1_b @ Wk
    wv_eff = ln1_g[None, :, None] * Wv
    bv_eff = bv + ln1_b @ Wv
    lw = np.exp(level_logits - level_logits.max())
    lw = lw / lw.sum()
    wo_eff = lw[:, None, None] * Wo
    boc_eff = np.einsum("l,lh->h", lw, bo) + np.einsum("lf,lfh->h", bv_eff, wo_eff)

    xn1 = _ln(x.astype(np.float64))                            # LN1 sans gamma/beta
    xn1_T32 = np.ascontiguousarray(
        np.swapaxes(xn1.astype(np.float32), 1, 2))             # [B,H,S]
    mbias = (1.0 - mask.astype(np.float32)) * np.float32(-1e9)

    in_maps = []
    for c in range(NCORES):
        b, g = c // 4, c % 4
        l, hb = g // 2, g % 2
        fs = slice(hb * FB, (hb + 1) * FB)
        in_maps.append({
            "xnT": xn1_T32[b].astype(bf16),
            "wq": np.ascontiguousarray(wq_eff[l][:, fs].astype(np.float32)).astype(bf16),
            "wk": np.ascontiguousarray(wk_eff[l][:, fs].astype(np.float32)).astype(bf16),
            "wv": np.ascontiguousarray(wv_eff[l][:, fs].astype(np.float32)).astype(bf16),
            "wo": np.ascontiguousarray(wo_eff[l][fs, :].astype(np.float32)).astype(bf16),
            "bqc": _colt(bq_eff[l][fs]),
            "bkc": _colt(bk_eff[l][fs]),
            "mb": _colt(mbias[b]),
        })

    import time as _time
    nc_a = _get_attn()
    t0 = _time.time()
    res_a = run_bass_kernel_spmd(nc_a, in_maps, core_ids=list(range(NCORES)))
    _PERF["a_wall_s"] = _time.time() - t0

    # device xres = x + sum of partials + boc
    xres = x.reshape(-1, H).copy()
    for c in range(NCORES):
        b = c // 4
        xres[b * S:(b + 1) * S] += res_a.results[c]["po"]
    xres += boc_eff.astype(np.float32)[None, :]

    # ---- host: exact fp32 attention -> routing (LN2, router, top-2) ----
    t0 = _time.time()
    xn1h = (xn1 * ln1_g + ln1_b).astype(np.float32).reshape(-1, H)
    attn_h = np.zeros((B * S, H), np.float32)
    lw32 = lw.astype(np.float32)
    for l in range(L):
        q = (xn1h @ Wq[l].astype(np.float32) + bq[l].astype(np.float32)) \
            .reshape(B, S, NH, DH) * np.float32(scale)
        k = (xn1h @ Wk[l].astype(np.float32) + bk[l].astype(np.float32)) \
            .reshape(B, S, NH, DH)
        v = (xn1h @ Wv[l].astype(np.float32) + bv[l].astype(np.float32)) \
            .reshape(B, S, NH, DH)
        ol = np.empty((B, S, H), np.float32)
        for b_ in range(B):
            mrow = mbias[b_][None, :]
            for n in range(NH):
                sc = q[b_, :, n] @ k[b_, :, n].T + mrow
                sc -= sc.max(-1, keepdims=True)
                e = np.exp(sc)
                a = e / e.sum(-1, keepdims=True)
                ol[b_, :, n * DH:(n + 1) * DH] = a @ v[b_, :, n]
        attn_h += lw32[l] * (
            ol.reshape(-1, H) @ Wo[l].astype(np.float32) + bo[l].astype(np.float32))
    xres_h = x.reshape(-1, H) + attn_h
    xn2 = (_ln(xres_h.astype(np.float64)) * ln2_g + ln2_b)     # [T,H] fp64
    logits = xn2 @ Wr + br
    pm = logits.max(-1, keepdims=True)
    probs = np.exp(logits - pm)
    probs /= probs.sum(-1, keepdims=True)
    order = np.argsort(-probs, axis=-1, kind="stable")
    topi = order[:, :TOPK]
    topv = np.take_along_axis(probs, topi, axis=-1)
    gates = topv / topv.sum(-1, keepdims=True)                 # [T,2]
    _PERF["host_route_s"] = _time.time() - t0

    tok_idx, gate_val = [], []
    for e in range(E):
        sel = np.nonzero(topi == e)
        tok_idx.append(sel[0])
        gate_val.append(gates[sel[0], sel[1]])
    counts = [len(t) for t in tok_idx]
    C = max(512, ((max(counts) + P - 1) // P) * P)
    while True:
        nch = (C + 511) // 512
        if C % nch == 0 and C // nch >= 256:
            break
        C += P

    w1f = ln2_g[None, :, None] * W1                            # [E,H,F]
    b1f = b1 + ln2_b @ W1                                      # [E,F]
    xn2_32 = xn2.astype(np.float32)
    e4 = ml_dtypes.float8_e4m3

    in_maps_b = []
    for e in range(E):
        xe = np.zeros((C, H), np.float32)
        xe[:counts[e]] = xn2_32[tok_idx[e]]
        xmq, xrq = _q8(np.ascontiguousarray(xe.T), SX)         # [H,C] fp8
        w1mq, w1rq = _q8(w1f[e], SW)
        w2mq, w2rq = _q8(W2[e], SW)
        g = np.zeros((C,), np.float32)
        g[:counts[e]] = gate_val[e].astype(np.float32) / np.float32(SW)
        in_maps_b.append({
            "xm": xmq, "xr": xrq,
            "w1m": np.ascontiguousarray(w1mq.astype(e4)),
            "w1r": np.ascontiguousarray(w1rq.astype(e4)),
            "w2m": np.ascontiguousarray(w2mq.astype(e4)),
            "w2r": np.ascontiguousarray(w2rq.astype(e4)),
            "b1c": _colt(b1f[e]),
            "gsc": np.ascontiguousarray(g.reshape(-1, P).T),
            "y": None,
        })
    for m in in_maps_b:
        del m["y"]

    nc_b = _get_expert(C)
    t0 = _time.time()
    res_b = run_bass_kernel_spmd(nc_b, in_maps_b, core_ids=list(range(NCORES)))
    _PERF["b_wall_s"] = _time.time() - t0
    _PERF["capacity"] = C
    _PERF["counts"] = counts

    out = xres
    for e in range(E):
        if counts[e]:
            out[tok_idx[e]] += res_b.results[e]["y"][:counts[e]]
    # b2 contribution: sum_e gate_e * b2[e]
    gmat = np.zeros((B * S, E), np.float64)
    np.put_along_axis(gmat, topi, gates, axis=-1)
    out += (gmat @ b2).astype(np.float32)
    return out.reshape(B, S, H).astype(np.float32)


# revision 26
# speedup vs baseline: 1.1237x; 1.1237x over previous
"""Trainium2 Bass kernel for nn_MoEMLABlock (MoE + multi-level attention block).

Strategy (8 NeuronCores, full inputs in / full output out):
  Launch A (head-parallel attention): 64 attention instances (level l,
    batch b, head h) are split as: core c -> batch b=c//4, level l=(c%4)//2,
    head block hb=(c%4)%2 (8 heads / 512 feature cols).  Each projection is
    computed exactly once across cores (no K/V recompute).  LayerNorm 1,
    the 1/sqrt(DH) scale and the level-softmax weight are folded into the
    projection weights on the host; V carries no bias (its effect, plus bo,
    is the constant `boc` added on the host).  Device math is bf16 inputs
    with fp32 PSUM accumulation.  Each core emits its partial attention
    output [S, H] token-major; host sums partials + residual.
  Host: exact fp32 attention recompute (BLAS) for ROUTING ONLY -- router
    logits, top-2 and gates are bit-robust against device rounding (the
    tightest p2/p3 logit gap in this data is 3e-5, so routing must not
    depend on device numerics).  Also LN2 + expert input prep.
  Launch B (expert-parallel FFN): core e runs expert e over its routed
    tokens with error-compensated fp8 (e4m3) DoubleRow matmuls:
    x@W1 ~= xm@W1m + xr@W1m + xm@W1r where *m/*r are fp8 main/residual
    parts (measured output error 5e-4 rel, 4x PE throughput per term).
    Gates are applied on device via the activation scale; the b2 bias is
    applied on the host (gates @ b2).
  Host: scatter-add combine + residuals.
"""

import numpy as np

H = 1024
NH = 16
DH = 64
L = 2
E = 8
FF = 4096
B = 2
S = 1024
EPS = 1e-5
P = 128
NCORES = 8
KO = H // P            # 8 contraction chunks over H
TT = S // P            # 8 token tiles
FB = 512               # feature block per core (8 heads)
MF = FF // P           # 32
TOPK = 2

SX = 16.0              # fp8 scale for expert input x
SW = 64.0              # fp8 scale for expert weights
SP = 4.0               # fp8 scale for attention exp-scores
SV = 16.0              # fp8 scale for attention V

_CACHE = {}
_PERF = {}


def _build_attn():
    """Launch A: one (batch, level, 8-head block) per core."""
    import concourse.bacc as bacc
    import concourse.mybir as mybir
    import concourse.tile as tile

    F32, F32R, BF16 = mybir.dt.float32, mybir.dt.float32r, mybir.dt.bfloat16
    F8 = mybir.dt.float8e4
    AF = mybir.ActivationFunctionType
    DR = mybir.MatmulPerfMode.DoubleRow

    nc = bacc.Bacc()
    xnT_h = nc.dram_tensor("xnT", [H, S], BF16, kind="ExternalInput")   # LN1(x_b)^T
    wq_h = nc.dram_tensor("wq", [H, FB], BF16, kind="ExternalInput")
    wk_h = nc.dram_tensor("wk", [H, FB], BF16, kind="ExternalInput")
    wv_h = nc.dram_tensor("wv", [H, FB], BF16, kind="ExternalInput")
    wo_h = nc.dram_tensor("wo", [FB, H], BF16, kind="ExternalInput")
    bq_h = nc.dram_tensor("bqc", [P, 4], F32, kind="ExternalInput")
    bk_h = nc.dram_tensor("bkc", [P, 4], F32, kind="ExternalInput")
    mb_h = nc.dram_tensor("mb", [P, TT], F32, kind="ExternalInput")     # key mask bias + ln(SP)
    po_h = nc.dram_tensor("po", [S, H], BF16, kind="ExternalOutput")    # partial attn out

    with tile.TileContext(nc) as tc:
        with tc.tile_pool(name="consts", bufs=1) as consts, \
             tc.tile_pool(name="big", bufs=1) as big, \
             tc.tile_pool(name="work", bufs=3) as work, \
             tc.tile_pool(name="ps_mm", bufs=2, space="PSUM") as ps_mm, \
             tc.tile_pool(name="ps_sc", bufs=2, space="PSUM") as ps_sc, \
             tc.tile_pool(name="ps_cx", bufs=2, space="PSUM") as ps_cx:

            ones_f = consts.tile([P, 1], F32)
            nc.vector.memset(ones_f[:], 1.0)
            ones_row = consts.tile([1, P], F32R)
            nc.vector.tensor_copy(ones_row[:], ones_f[:1, :].to_broadcast((1, P)))

            bq_t = consts.tile([P, 4], F32)
            nc.sync.dma_start(bq_t[:], bq_h[:])
            bk_t = consts.tile([P, 4], F32)
            nc.sync.dma_start(bk_t[:], bk_h[:])
            mb_t = consts.tile([P, TT], F32)
            nc.sync.dma_start(mb_t[:], mb_h[:])

            # Two DMA queues (SP + Act): V inputs on SP (first use), the
            # rest in parallel on the Act queue.
            wv_t = big.tile([P, KO, FB], BF16)
            nc.sync.dma_start(wv_t[:], wv_h[:].rearrange("(ko p) f -> p ko f", p=P))
            xn_a = big.tile([P, KO, FB], BF16)     # tokens 0..511
            nc.sync.dma_start(
                xn_a[:], xnT_h[:, :FB].rearrange("(ko p) t -> p ko t", p=P))
            xn_b = big.tile([P, KO, FB], BF16)     # tokens 512..1023
            nc.scalar.dma_start(
                xn_b[:], xnT_h[:, FB:].rearrange("(ko p) t -> p ko t", p=P))
            wq_t = big.tile([P, KO, FB], BF16)
            nc.scalar.dma_start(wq_t[:], wq_h[:].rearrange("(ko p) f -> p ko f", p=P))
            wk_t = big.tile([P, KO, FB], BF16)
            nc.scalar.dma_start(wk_t[:], wk_h[:].rearrange("(ko p) f -> p ko f", p=P))
            wo_t = big.tile([P, 4, H], BF16)
            nc.scalar.dma_start(wo_t[:], wo_h[:].rearrange("(fc p) h -> p fc h", p=P))

            def xntok(t0):
                # [P, KO, 128] token block starting at t0
                src = xn_a if t0 < FB else xn_b
                o = t0 % FB
                return src[:, :, o:o + P]

            def xnq(qc):
                return xn_a if qc == 0 else xn_b

            # ---- tiles; emission order below keeps the Act exp chain (the
            #      pacer) fed from early on while PE fills with proj work ----
            v_t = big.tile([P, TT, 8, DH + 2], F8)
            nc.vector.memset(v_t[:, :, :, DH:DH + 1], 1.0)
            nc.vector.memset(v_t[:, :, :, DH + 1:DH + 2], 0.0)
            q_t = big.tile([P, 4, S], BF16)
            k_t = big.tile([P, 4, S], BF16)
            ctx = big.tile([P, 4, S], BF16)
            p_sbs = {}

            def emit_v(half):
                for tt in range(half * 4, half * 4 + 4):
                    vps = ps_mm.tile([P, FB], F32, tag="mm")
                    for kc in range(KO):
                        nc.tensor.matmul(
                            vps[:], xntok(tt * P)[:, kc, :], wv_t[:, kc, :],
                            start=(kc == 0), stop=(kc == KO - 1),
                        )
                    nc.vector.tensor_scalar_mul(
                        v_t[:, tt, :, 0:DH],
                        vps[:].rearrange("p (h d) -> p h d", d=DH),
                        float(SV),
                    )

            def emit_qk(ft):
                for dst, wsrc, bsrc in ((q_t, wq_t, bq_t), (k_t, wk_t, bk_t)):
                    for qc in range(2):
                        pps = ps_mm.tile([P, FB], F32, tag="mm")
                        for kc in range(KO):
                            nc.tensor.matmul(
                                pps[:],
                                wsrc[:, kc, ft * P:(ft + 1) * P],
                                xnq(qc)[:, kc, :],
                                start=(kc == 0), stop=(kc == KO - 1),
                            )
                        nc.vector.tensor_scalar_add(
                            dst[:, ft, qc * FB:(qc + 1) * FB], pps[:],
                            bsrc[:, ft:ft + 1],
                        )

            def emit_sc(h):
                ft, pb = h // 2, (h % 2) * DH
                p_sb = work.tile([P, TT, S], F8, tag="p_sb")
                p_sbs[h] = p_sb
                for tt in range(TT):
                    sps = ps_sc.tile([P, S], F32, tag="sc")
                    for qc in range(2):
                        nc.tensor.matmul(
                            sps[:, qc * FB:(qc + 1) * FB],
                            k_t[pb:pb + DH, ft, tt * P:(tt + 1) * P],
                            q_t[pb:pb + DH, ft, qc * FB:(qc + 1) * FB],
                            start=True, stop=True,
                        )
                    nc.scalar.activation(
                        p_sb[:, tt, :], sps[:],
                        AF.Exp, bias=mb_t[:, tt:tt + 1],
                    )

            def emit_ctx(h):
                ft, pb = h // 2, (h % 2) * DH
                p_sb = p_sbs[h]
                for qc in range(2):
                    cxps = ps_cx.tile([DH + 2, FB], F32, tag="cx")
                    for t2 in range(TT // 2):
                        nc.tensor.matmul(
                            cxps[:],
                            v_t[:, 2 * t2:2 * t2 + 2, h, :],
                            p_sb[:, 2 * t2:2 * t2 + 2, qc * FB:(qc + 1) * FB],
                            start=(t2 == 0), stop=(t2 == TT // 2 - 1),
                            perf_mode=DR,
                        )
                    r = work.tile([1, FB], F32R, tag="r")
                    with nc.allow_low_precision(reason="softmax recip feeds broadcast matmul"):
                        nc.vector.reciprocal(r[:], cxps[DH:DH + 1, :])
                    rbps = ps_mm.tile([P, FB], F32, tag="mm")
                    nc.tensor.matmul(rbps[:DH, :], ones_row[:, :DH], r[:], start=True, stop=True)
                    rb_sb = work.tile([DH, FB], F32, tag="rb_sb")
                    nc.vector.tensor_copy(rb_sb[:], rbps[:DH, :])
                    nc.vector.tensor_mul(
                        ctx[pb:pb + DH, ft, qc * FB:(qc + 1) * FB],
                        cxps[0:DH, :], rb_sb[:],
                    )

            emit_v(0)
            emit_qk(0)
            emit_sc(0)
            emit_v(1)
            emit_sc(1)
            emit_qk(1)
            emit_sc(2)
            emit_ctx(0)
            emit_sc(3)
            emit_ctx(1)
            emit_qk(2)
            emit_sc(4)
            emit_ctx(2)
            emit_sc(5)
            emit_ctx(3)
            emit_qk(3)
            emit_sc(6)
            emit_ctx(4)
            emit_sc(7)
            emit_ctx(5)
            emit_ctx(6)
            emit_ctx(7)

            # ---- O-projection straight to token-major, DMA out ----
            po_t = big.tile([P, TT, H], BF16)
            for tt in range(TT):
                for hc in range(2):
                    ops = ps_mm.tile([P, FB], F32, tag="mm")
                    for fc in range(4):
                        nc.tensor.matmul(
                            ops[:],
                            ctx[:, fc, tt * P:(tt + 1) * P],
                            wo_t[:, fc, hc * FB:(hc + 1) * FB],
                            start=(fc == 0), stop=(fc == 3),
                        )
                    nc.vector.tensor_copy(po_t[:, tt, hc * FB:(hc + 1) * FB], ops[:])
                nc.sync.dma_start(
                    po_h[:].rearrange("(tt p) h -> p tt h", p=P)[:, tt, :],
                    po_t[:, tt, :],
                )

    nc.finalize()
    return nc


def _build_expert(C):
    """Launch B: one expert FFN over C routed tokens, compensated fp8."""
    import concourse.bacc as bacc
    import concourse.mybir as mybir
    import concourse.tile as tile

    F32, F8 = mybir.dt.float32, mybir.dt.float8e4
    AF = mybir.ActivationFunctionType
    DR = mybir.MatmulPerfMode.DoubleRow

    assert C % P == 0 and C >= 512, C
    NCH = max(1, (C + 511) // 512)
    assert C % NCH == 0, (C, NCH)
    CN = C // NCH
    assert 256 <= CN <= 512, CN
    CT = C // P

    nc = bacc.Bacc()
    xm_h = nc.dram_tensor("xm", [H, C], F8, kind="ExternalInput")   # SX*x main
    xr_h = nc.dram_tensor("xr", [H, C], F8, kind="ExternalInput")   # SX*x residual
    w1m_h = nc.dram_tensor("w1m", [H, FF], F8, kind="ExternalInput")
    w1r_h = nc.dram_tensor("w1r", [H, FF], F8, kind="ExternalInput")
    w2m_h = nc.dram_tensor("w2m", [FF, H], F8, kind="ExternalInput")
    w2r_h = nc.dram_tensor("w2r", [FF, H], F8, kind="ExternalInput")
    b1_h = nc.dram_tensor("b1c", [P, MF], F32, kind="ExternalInput")
    gs_h = nc.dram_tensor("gsc", [P, CT], F32, kind="ExternalInput")  # gate/SW per token
    y_h = nc.dram_tensor("y", [C, H], F32, kind="ExternalOutput")

    with tile.TileContext(nc) as tc:
        with tc.tile_pool(name="consts", bufs=1) as consts, \
             tc.tile_pool(name="big", bufs=1) as big, \
             tc.tile_pool(name="w1s", bufs=3) as w1s, \
             tc.tile_pool(name="ht", bufs=2) as htp, \
             tc.tile_pool(name="ps_mm", bufs=3, space="PSUM") as ps_mm:

            b1_t = consts.tile([P, MF], F32)
            nc.sync.dma_start(b1_t[:], b1_h[:])
            gs_t = consts.tile([P, CT], F32)
            nc.sync.dma_start(gs_t[:], gs_h[:])

            xm_t = big.tile([P, KO, C], F8)
            nc.sync.dma_start(xm_t[:], xm_h[:].rearrange("(ko p) c -> p ko c", p=P))
            xr_t = big.tile([P, KO, C], F8)
            nc.sync.dma_start(xr_t[:], xr_h[:].rearrange("(ko p) c -> p ko c", p=P))
            # W2 loads interleaved into the W1 chunk stream (slack-scheduled)
            w2m_t = big.tile([P, MF, H], F8)
            w2r_t = big.tile([P, MF, H], F8)

            hm_t = big.tile([P, MF, C], F8)
            hr_t = big.tile([P, MF, C], F8)

            # ---- W1 pass: h = gelu((xm+xr)@(w1m+w1r)/(SX*SW) + b1) ----
            for mfc in range(FF // FB):            # 8 chunks of 512 cols
                w1m_c = w1s.tile([P, KO, FB], F8, tag="w1m")
                nc.sync.dma_start(
                    w1m_c[:],
                    w1m_h[:, mfc * FB:(mfc + 1) * FB].rearrange("(ko p) f -> p ko f", p=P),
                )
                w1r_c = w1s.tile([P, KO, FB], F8, tag="w1r")
                nc.sync.dma_start(
                    w1r_c[:],
                    w1r_h[:, mfc * FB:(mfc + 1) * FB].rearrange("(ko p) f -> p ko f", p=P),
                )
                if mfc == 3:
                    nc.sync.dma_start(
                        w2m_t[:], w2m_h[:].rearrange("(fc p) h -> p fc h", p=P))
                elif mfc == 5:
                    nc.sync.dma_start(
                        w2r_t[:], w2r_h[:].rearrange("(fc p) h -> p fc h", p=P))
                for mf in range(4):
                    mfg = mfc * 4 + mf
                    for nch in range(NCH):
                        cs = slice(nch * CN, (nch + 1) * CN)
                        hps = ps_mm.tile([P, 512], F32, tag="mm")
                        terms = [(w1m_c, xm_t), (w1m_c, xr_t), (w1r_c, xm_t)]
                        nterm = len(terms)
                        for ti, (wt, xt) in enumerate(terms):
                            for kc2 in range(KO // 2):
                                nc.tensor.matmul(
                                    hps[:, :CN],
                                    wt[:, 2 * kc2:2 * kc2 + 2, mf * P:(mf + 1) * P],
                                    xt[:, 2 * kc2:2 * kc2 + 2, cs],
                                    start=(ti == 0 and kc2 == 0),
                                    stop=(ti == nterm - 1 and kc2 == KO // 2 - 1),
                                    perf_mode=DR,
                                )
                        ht = htp.tile([P, 512], F32, tag="ht")
                        nc.scalar.activation(
                            ht[:, :CN], hps[:, :CN], AF.Gelu_apprx_tanh,
                            bias=b1_t[:, mfg:mfg + 1], scale=1.0 / (SX * SW),
                        )
                        # split the fp8 quantize-copy across Act/DVE
                        if mfg % 2 == 0:
                            nc.scalar.activation(hm_t[:, mfg, cs], ht[:, :CN], AF.Copy)
                        else:
                            nc.vector.tensor_copy(hm_t[:, mfg, cs], ht[:, :CN])
                        nc.vector.tensor_sub(hr_t[:, mfg, cs], ht[:, :CN], hm_t[:, mfg, cs])

            # ---- W2 pass: y = (h@(w2m+w2r))/SW * gate, token-major ----
            y_t = big.tile([P, CT, H], F32)
            for ct in range(CT):
                for hc in range(2):
                    yps = ps_mm.tile([P, 512], F32, tag="mm")
                    terms = [(hm_t, w2m_t), (hr_t, w2m_t), (hm_t, w2r_t)]
                    nterm = len(terms)
                    for ti, (ht_, wt) in enumerate(terms):
                        for fc2 in range(MF // 2):
                            nc.tensor.matmul(
                                yps[:],
                                ht_[:, 2 * fc2:2 * fc2 + 2, ct * P:(ct + 1) * P],
                                wt[:, 2 * fc2:2 * fc2 + 2, hc * FB:(hc + 1) * FB],
                                start=(ti == 0 and fc2 == 0),
                                stop=(ti == nterm - 1 and fc2 == MF // 2 - 1),
                                perf_mode=DR,
                            )
                    nc.scalar.activation(
                        y_t[:, ct, hc * FB:(hc + 1) * FB], yps[:],
                        AF.Copy, scale=gs_t[:, ct:ct + 1],
                    )
                nc.sync.dma_start(
                    y_h[:].rearrange("(ct p) h -> p ct h", p=P)[:, ct, :],
                    y_t[:, ct, :],
                )

    nc.finalize()
    return nc


def _get_attn():
    if "attn" not in _CACHE:
        _CACHE["attn"] = _build_attn()
    return _CACHE["attn"]


def _get_expert(C):
    key = ("exp", C)
    if key not in _CACHE:
        _CACHE[key] = _build_expert(C)
    return _CACHE[key]


def _ln(x):
    m = x.mean(-1, keepdims=True)
    v = x.var(-1, keepdims=True)
    return (x - m) / np.sqrt(v + EPS)


def _q8(a, s):
    """e4m3 quantize a*s, return (main, residual) as raw scaled fp8 arrays."""
    import ml_dtypes
    e4 = ml_dtypes.float8_e4m3
    m = (a * s).astype(np.float32).astype(e4)
    r = ((a * s).astype(np.float32) - m.astype(np.float32)).astype(e4)
    return m, r


def _colt(vec):
    v32 = np.ascontiguousarray(np.asarray(vec, dtype=np.float32))
    return np.ascontiguousarray(v32.reshape(-1, P).T)


def kernel(**inputs):
    import ml_dtypes
    from concourse.bass_utils import run_bass_kernel_spmd

    bf16 = ml_dtypes.bfloat16
    f = lambda k: np.asarray(inputs[k], dtype=np.float32)
    x = f("hidden_states")                        # [B, S, H]
    mask = np.asarray(inputs["attention_mask"])   # [B, S] int32
    ln1_g, ln1_b = f("ln1_g").astype(np.float64), f("ln1_b").astype(np.float64)
    ln2_g, ln2_b = f("ln2_g").astype(np.float64), f("ln2_b").astype(np.float64)
    Wq, Wk, Wv, Wo = (f(k).astype(np.float64) for k in ("Wq", "Wk", "Wv", "Wo"))
    bq, bk, bv, bo = (f(k).astype(np.float64) for k in ("bq", "bk", "bv", "bo"))
    level_logits = f("level_logits").astype(np.float64)
    Wr, br = f("Wr").astype(np.float64), f("br").astype(np.float64)
    W1, b1 = f("W1").astype(np.float64), f("b1").astype(np.float64)
    W2, b2 = f("W2").astype(np.float64), f("b2").astype(np.float64)

    # ---- host folding (as in reference, with LN1 gamma/beta absorbed) ----
    scale = 1.0 / np.sqrt(DH)
    wq_eff = (ln1_g[None, :, None] * Wq) * scale               # [L,H,H]
    bq_eff = (bq + ln1_b @ Wq) * scale                         # [L,H]
    wk_eff = ln1_g[None, :, None] * Wk
    bk_eff = bk + ln1_b @ Wk
    wv_eff = ln1_g[None, :, None] * Wv
    bv_eff = bv + ln1_b @ Wv
    lw = np.exp(level_logits - level_logits.max())
    lw = lw / lw.sum()
    wo_eff = lw[:, None, None] * Wo
    boc_eff = np.einsum("l,lh->h", lw, bo) + np.einsum("lf,lfh->h", bv_eff, wo_eff)
    wo_dev = wo_eff / SV          # device ctx carries a factor of SV

    xn1 = _ln(x.astype(np.float64))                            # LN1 sans gamma/beta
    xn1_T32 = np.ascontiguousarray(
        np.swapaxes(xn1.astype(np.float32), 1, 2))             # [B,H,S]
    mbias = (1.0 - mask.astype(np.float32)) * np.float32(-1e9)
    mb_dev = mbias + np.float32(np.log(SP))                    # exp out pre-scaled by SP

    in_maps = []
    for c in range(NCORES):
        b, g = c // 4, c % 4
        l, hb = g // 2, g % 2
        fs = slice(hb * FB, (hb + 1) * FB)
        in_maps.append({
            "xnT": xn1_T32[b].astype(bf16),
            "wq": np.ascontiguousarray(wq_eff[l][:, fs].astype(np.float32)).astype(bf16),
            "wk": np.ascontiguousarray(wk_eff[l][:, fs].astype(np.float32)).astype(bf16),
            "wv": np.ascontiguousarray(wv_eff[l][:, fs].astype(np.float32)).astype(bf16),
            "wo": np.ascontiguousarray(wo_dev[l][fs, :].astype(np.float32)).astype(bf16),
            "bqc": _colt(bq_eff[l][fs]),
            "bkc": _colt(bk_eff[l][fs]),
            "mb": _colt(mb_dev[b]),
        })

    import time as _time
    nc_a = _get_attn()
    t0 = _time.time()
    res_a = run_bass_kernel_spmd(nc_a, in_maps, core_ids=list(range(NCORES)))
    _PERF["a_wall_s"] = _time.time() - t0

    # device xres = x + sum of partials + boc
    xres = x.reshape(-1, H).copy()
    for c in range(NCORES):
        b = c // 4
        xres[b * S:(b + 1) * S] += res_a.results[c]["po"].astype(np.float32)
    xres += boc_eff.astype(np.float32)[None, :]

    # ---- host: exact fp32 attention -> routing (LN2, router, top-2) ----
    t0 = _time.time()
    xn1h = (xn1 * ln1_g + ln1_b).astype(np.float32).reshape(-1, H)
    attn_h = np.zeros((B * S, H), np.float32)
    lw32 = lw.astype(np.float32)
    for l in range(L):
        q = (xn1h @ Wq[l].astype(np.float32) + bq[l].astype(np.float32)) \
            .reshape(B, S, NH, DH) * np.float32(scale)
        k = (xn1h @ Wk[l].astype(np.float32) + bk[l].astype(np.float32)) \
            .reshape(B, S, NH, DH)
        v = (xn1h @ Wv[l].astype(np.float32) + bv[l].astype(np.float32)) \
            .reshape(B, S, NH, DH)
        ol = np.empty((B, S, H), np.float32)
        for b_ in range(B):
            mrow = mbias[b_][None, :]
            for n in range(NH):
                sc = q[b_, :, n] @ k[b_, :, n].T + mrow
                sc -= sc.max(-1, keepdims=True)
                e = np.exp(sc)
                a = e / e.sum(-1, keepdims=True)
                ol[b_, :, n * DH:(n + 1) * DH] = a @ v[b_, :, n]
        attn_h += lw32[l] * (
            ol.reshape(-1, H) @ Wo[l].astype(np.float32) + bo[l].astype(np.float32))
    xres_h = x.reshape(-1, H) + attn_h
    xn2 = (_ln(xres_h.astype(np.float64)) * ln2_g + ln2_b)     # [T,H] fp64
    logits = xn2 @ Wr + br
    pm = logits.max(-1, keepdims=True)
    probs = np.exp(logits - pm)
    probs /= probs.sum(-1, keepdims=True)
    order = np.argsort(-probs, axis=-1, kind="stable")
    topi = order[:, :TOPK]
    topv = np.take_along_axis(probs, topi, axis=-1)
    gates = topv / topv.sum(-1, keepdims=True)                 # [T,2]
    _PERF["host_route_s"] = _time.time() - t0

    tok_idx, gate_val = [], []
    for e in range(E):
        sel = np.nonzero(topi == e)
        tok_idx.append(sel[0])
        gate_val.append(gates[sel[0], sel[1]])
    counts = [len(t) for t in tok_idx]
    C = max(512, ((max(counts) + P - 1) // P) * P)
    while True:
        nch = (C + 511) // 512
        if C % nch == 0 and C // nch >= 256:
            break
        C += P

    w1f = ln2_g[None, :, None] * W1                            # [E,H,F]
    b1f = b1 + ln2_b @ W1                                      # [E,F]
    xn2_32 = xn2.astype(np.float32)
    e4 = ml_dtypes.float8_e4m3

    in_maps_b = []
    for e in range(E):
        xe = np.zeros((C, H), np.float32)
        xe[:counts[e]] = xn2_32[tok_idx[e]]
        xmq, xrq = _q8(np.ascontiguousarray(xe.T), SX)         # [H,C] fp8
        w1mq, w1rq = _q8(w1f[e], SW)
        w2mq, w2rq = _q8(W2[e], SW)
        g = np.zeros((C,), np.float32)
        g[:counts[e]] = gate_val[e].astype(np.float32) / np.float32(SW)
        in_maps_b.append({
            "xm": xmq, "xr": xrq,
            "w1m": np.ascontiguousarray(w1mq.astype(e4)),
            "w1r": np.ascontiguousarray(w1rq.astype(e4)),
            "w2m": np.ascontiguousarray(w2mq.astype(e4)),
            "w2r": np.ascontiguousarray(w2rq.astype(e4)),
            "b1c": _colt(b1f[e]),
            "gsc": np.ascontiguousarray(g.reshape(-1, P).T),
            "y": None,
        })
    for m in in_maps_b:
        del m["y"]

    nc_b = _get_expert(C)
    t0 = _time.time()
    res_b = run_bass_kernel_spmd(nc_b, in_maps_b, core_ids=list(range(NCORES)))
    _PERF["b_wall_s"] = _time.time() - t0
    _PERF["capacity"] = C
    _PERF["counts"] = counts

    out = xres
    for e in range(E):
        if counts[e]:
            out[tok_idx[e]] += res_b.results[e]["y"][:counts[e]]
    # b2 contribution: sum_e gate_e * b2[e]
    gmat = np.zeros((B * S, E), np.float64)
    np.put_along_axis(gmat, topi, gates, axis=-1)
    out += (gmat @ b2).astype(np.float32)
    return out.reshape(B, S, H).astype(np.float32)


# revision 38
# speedup vs baseline: 1.1874x; 1.0567x over previous
"""Trainium2 Bass kernel for nn_MoEMLABlock (MoE + multi-level attention block).

Strategy (8 NeuronCores, full inputs in / full output out):
  Launch A (head-parallel attention): 64 attention instances (level l,
    batch b, head h) are split as: core c -> batch b=c//4, level l=(c%4)//2,
    head block hb=(c%4)%2 (8 heads / 512 feature cols).  Each projection is
    computed exactly once across cores (no K/V recompute).  LayerNorm 1,
    the 1/sqrt(DH) scale and the level-softmax weight are folded into the
    projection weights on the host; V carries no bias (its effect, plus bo,
    is the constant `boc` added on the host).  Device math is bf16 inputs
    with fp32 PSUM accumulation.  Each core emits its partial attention
    output [S, H] token-major; host sums partials + residual.
  Host: exact fp32 attention recompute (BLAS) for ROUTING ONLY -- router
    logits, top-2 and gates are bit-robust against device rounding (the
    tightest p2/p3 logit gap in this data is 3e-5, so routing must not
    depend on device numerics).  Also LN2 + expert input prep.
  Launch B (expert-parallel FFN): core e runs expert e over its routed
    tokens with error-compensated fp8 (e4m3) DoubleRow matmuls:
    x@W1 ~= xm@W1m + xr@W1m + xm@W1r where *m/*r are fp8 main/residual
    parts (measured output error 5e-4 rel, 4x PE throughput per term).
    Gates are applied on device via the activation scale; the b2 bias is
    applied on the host (gates @ b2).
  Host: scatter-add combine + residuals.
"""

import numpy as np

H = 1024
NH = 16
DH = 64
L = 2
E = 8
FF = 4096
B = 2
S = 1024
EPS = 1e-5
P = 128
NCORES = 8
KO = H // P            # 8 contraction chunks over H
TT = S // P            # 8 token tiles
FB = 512               # feature block per core (8 heads)
MF = FF // P           # 32
TOPK = 2

SX = 16.0              # fp8 scale for expert input x
SW = 64.0              # fp8 scale for expert weights
SP = 4.0               # fp8 scale for attention exp-scores
SV = 16.0              # fp8 scale for attention V
SXA = 16.0             # fp8 scale for attention input LN1(x)
SWQ = 512.0            # fp8 scale for wq (carries the 1/sqrt(DH) fold)
SWK = 64.0             # fp8 scale for wk
SWV = 64.0             # fp8 scale for wv

_CACHE = {}
_PERF = {}


def _build_attn():
    """Launch A: one (batch, level, 8-head block) per core."""
    import concourse.bacc as bacc
    import concourse.mybir as mybir
    import concourse.tile as tile

    F32, F32R, BF16 = mybir.dt.float32, mybir.dt.float32r, mybir.dt.bfloat16
    F8 = mybir.dt.float8e4
    AF = mybir.ActivationFunctionType
    DR = mybir.MatmulPerfMode.DoubleRow

    nc = bacc.Bacc()
    xam_h = nc.dram_tensor("xam", [H, S], F8, kind="ExternalInput")     # SXA*LN1(x_b)^T main
    xar_h = nc.dram_tensor("xar", [H, S], F8, kind="ExternalInput")     # ... residual
    wqm_h = nc.dram_tensor("wqm", [H, FB], F8, kind="ExternalInput")
    wqr_h = nc.dram_tensor("wqr", [H, FB], F8, kind="ExternalInput")
    wkm_h = nc.dram_tensor("wkm", [H, FB], F8, kind="ExternalInput")
    wkr_h = nc.dram_tensor("wkr", [H, FB], F8, kind="ExternalInput")
    wvm_h = nc.dram_tensor("wvm", [H, FB], F8, kind="ExternalInput")
    wvr_h = nc.dram_tensor("wvr", [H, FB], F8, kind="ExternalInput")
    wo_h = nc.dram_tensor("wo", [FB, H], BF16, kind="ExternalInput")
    bq_h = nc.dram_tensor("bqc", [P, 4], F32, kind="ExternalInput")
    bk_h = nc.dram_tensor("bkc", [P, 4], F32, kind="ExternalInput")
    mb_h = nc.dram_tensor("mb", [P, TT], F32, kind="ExternalInput")     # key mask bias + ln(SP)
    po_h = nc.dram_tensor("po", [S, H], BF16, kind="ExternalOutput")    # partial attn out

    with tile.TileContext(nc) as tc:
        with tc.tile_pool(name="consts", bufs=1) as consts, \
             tc.tile_pool(name="big", bufs=1) as big, \
             tc.tile_pool(name="work", bufs=3) as work, \
             tc.tile_pool(name="ps_mm", bufs=2, space="PSUM") as ps_mm, \
             tc.tile_pool(name="ps_sc", bufs=2, space="PSUM") as ps_sc, \
             tc.tile_pool(name="ps_cx", bufs=2, space="PSUM") as ps_cx:

            ones_f = consts.tile([P, 1], F32)
            nc.vector.memset(ones_f[:], 1.0)
            ones_row = consts.tile([1, P], F32R)
            nc.vector.tensor_copy(ones_row[:], ones_f[:1, :].to_broadcast((1, P)))

            bq_t = consts.tile([P, 4], F32)
            nc.sync.dma_start(bq_t[:], bq_h[:])
            bk_t = consts.tile([P, 4], F32)
            nc.sync.dma_start(bk_t[:], bk_h[:])
            mb_t = consts.tile([P, TT], F32)
            nc.sync.dma_start(mb_t[:], mb_h[:])

            # Two DMA queues (SP + Act): Q/K path inputs on SP in first-use
            # order; V weights + wo in parallel on the Act queue.
            def ld(dst, src, eng):
                eng.dma_start(dst[:], src.rearrange("(ko p) f -> p ko f", p=P))

            wqm_t = big.tile([P, KO, FB], F8)
            ld(wqm_t, wqm_h[:], nc.sync)
            wqr_t = big.tile([P, KO, FB], F8)
            ld(wqr_t, wqr_h[:], nc.sync)
            xm_a = big.tile([P, KO, FB], F8)       # tokens 0..511
            ld(xm_a, xam_h[:, :FB], nc.sync)
            xr_a = big.tile([P, KO, FB], F8)
            ld(xr_a, xar_h[:, :FB], nc.sync)
            xm_b = big.tile([P, KO, FB], F8)       # tokens 512..1023
            ld(xm_b, xam_h[:, FB:], nc.sync)
            xr_b = big.tile([P, KO, FB], F8)
            ld(xr_b, xar_h[:, FB:], nc.sync)
            wkm_t = big.tile([P, KO, FB], F8)
            ld(wkm_t, wkm_h[:], nc.sync)
            wkr_t = big.tile([P, KO, FB], F8)
            ld(wkr_t, wkr_h[:], nc.sync)
            wvm_t = big.tile([P, KO, FB], F8)
            ld(wvm_t, wvm_h[:], nc.scalar)
            wvr_t = big.tile([P, KO, FB], F8)
            ld(wvr_t, wvr_h[:], nc.scalar)
            wo_t = big.tile([P, 4, H], BF16)
            nc.scalar.dma_start(wo_t[:], wo_h[:].rearrange("(fc p) h -> p fc h", p=P))

            def xtok(src, t0):
                # [P, KO, 128] token block starting at t0 (main or residual)
                return src[:, :, t0 % FB:t0 % FB + P]

            # ---- tiles; emission order below keeps the Act exp chain (the
            #      pacer) fed from early on while PE fills with proj work ----
            v_t = big.tile([P, TT, 8, DH + 2], F8)
            nc.vector.memset(v_t[:, :, :, DH:DH + 1], 1.0)
            nc.vector.memset(v_t[:, :, :, DH + 1:DH + 2], 0.0)
            q_t = big.tile([P, 4, S], BF16)
            k_t = big.tile([P, 4, S], BF16)
            ctx = big.tile([P, 4, S], BF16)
            p_sbs = {}

            def comp_mm(out_ap, x_of, w_of, x_is_lhs):
                """3-term compensated fp8 DoubleRow accumulation group:
                xm@wm + xr@wm + xm@wr."""
                terms = [(0, 0), (1, 0), (0, 1)]   # (x residual?, w residual?)
                for ti, (xi, wi) in enumerate(terms):
                    for kc2 in range(KO // 2):
                        ks = slice(2 * kc2, 2 * kc2 + 2)
                        xa, wa = x_of(xi, ks), w_of(wi, ks)
                        lhsT, rhs = (xa, wa) if x_is_lhs else (wa, xa)
                        nc.tensor.matmul(
                            out_ap, lhsT, rhs,
                            start=(ti == 0 and kc2 == 0),
                            stop=(ti == 2 and kc2 == KO // 2 - 1),
                            perf_mode=DR,
                        )

            def emit_v(half):
                for tt in range(half * 4, half * 4 + 4):
                    xmx = xm_a if tt < 4 else xm_b
                    xrx = xr_a if tt < 4 else xr_b
                    vps = ps_mm.tile([P, FB], F32, tag="mm")
                    comp_mm(
                        vps[:],
                        lambda xi, ks: xtok(xrx if xi else xmx, tt * P)[:, ks, :],
                        lambda wi, ks: (wvr_t if wi else wvm_t)[:, ks, :],
                        x_is_lhs=True,
                    )
                    nc.vector.tensor_scalar_mul(
                        v_t[:, tt, :, 0:DH],
                        vps[:].rearrange("p (h d) -> p h d", d=DH),
                        float(SV / (SXA * SWV)),
                    )

            def emit_qk(ft):
                for dst, wm, wr, bsrc, sw in (
                        (q_t, wqm_t, wqr_t, bq_t, SWQ),
                        (k_t, wkm_t, wkr_t, bk_t, SWK)):
                    for qc in range(2):
                        xmx = xm_a if qc == 0 else xm_b
                        xrx = xr_a if qc == 0 else xr_b
                        pps = ps_mm.tile([P, FB], F32, tag="mm")
                        comp_mm(
                            pps[:],
                            lambda xi, ks: (xrx if xi else xmx)[:, ks, :],
                            lambda wi, ks: (wr if wi else wm)[:, ks, ft * P:(ft + 1) * P],
                            x_is_lhs=False,
                        )
                        nc.vector.tensor_scalar(
                            dst[:, ft, qc * FB:(qc + 1) * FB], pps[:],
                            float(1.0 / (SXA * sw)), bsrc[:, ft:ft + 1],
                            op0=mybir.AluOpType.mult, op1=mybir.AluOpType.add,
                        )

            def emit_sc(h):
                ft, pb = h // 2, (h % 2) * DH
                p_sb = work.tile([P, TT, S], F8, tag="p_sb")
                p_sbs[h] = p_sb
                for tt in range(TT):
                    sps = ps_sc.tile([P, S], F32, tag="sc")
                    for qc in range(2):
                        nc.tensor.matmul(
                            sps[:, qc * FB:(qc + 1) * FB],
                            k_t[pb:pb + DH, ft, tt * P:(tt + 1) * P],
                            q_t[pb:pb + DH, ft, qc * FB:(qc + 1) * FB],
                            start=True, stop=True,
                        )
                    nc.scalar.activation(
                        p_sb[:, tt, :], sps[:],
                        AF.Exp, bias=mb_t[:, tt:tt + 1],
                    )

            def emit_ctx(h):
                ft, pb = h // 2, (h % 2) * DH
                p_sb = p_sbs[h]
                for qc in range(2):
                    cxps = ps_cx.tile([DH + 2, FB], F32, tag="cx")
                    for t2 in range(TT // 2):
                        nc.tensor.matmul(
                            cxps[:],
                            v_t[:, 2 * t2:2 * t2 + 2, h, :],
                            p_sb[:, 2 * t2:2 * t2 + 2, qc * FB:(qc + 1) * FB],
                            start=(t2 == 0), stop=(t2 == TT // 2 - 1),
                            perf_mode=DR,
                        )
                    r = work.tile([1, FB], F32R, tag="r")
                    with nc.allow_low_precision(reason="softmax recip feeds broadcast matmul"):
                        nc.vector.reciprocal(r[:], cxps[DH:DH + 1, :])
                    rbps = ps_mm.tile([P, FB], F32, tag="mm")
                    nc.tensor.matmul(rbps[:DH, :], ones_row[:, :DH], r[:], start=True, stop=True)
                    rb_sb = work.tile([DH, FB], F32, tag="rb_sb")
                    nc.vector.tensor_copy(rb_sb[:], rbps[:DH, :])
                    nc.vector.tensor_mul(
                        ctx[pb:pb + DH, ft, qc * FB:(qc + 1) * FB],
                        cxps[0:DH, :], rb_sb[:],
                    )

            emit_qk(0)
            emit_sc(0)
            emit_sc(1)
            emit_v(0)
            emit_qk(1)
            emit_sc(2)
            emit_v(1)
            emit_ctx(0)
            emit_sc(3)
            emit_ctx(1)
            emit_qk(2)
            emit_sc(4)
            emit_ctx(2)
            emit_sc(5)
            emit_ctx(3)
            emit_qk(3)
            emit_sc(6)
            emit_ctx(4)
            emit_sc(7)
            emit_ctx(5)
            emit_ctx(6)
            emit_ctx(7)

            # ---- O-projection straight to token-major, DMA out ----
            po_t = big.tile([P, TT, H], BF16)
            for tt in range(TT):
                for hc in range(2):
                    ops = ps_mm.tile([P, FB], F32, tag="mm")
                    for fc in range(4):
                        nc.tensor.matmul(
                            ops[:],
                            ctx[:, fc, tt * P:(tt + 1) * P],
                            wo_t[:, fc, hc * FB:(hc + 1) * FB],
                            start=(fc == 0), stop=(fc == 3),
                        )
                    nc.vector.tensor_copy(po_t[:, tt, hc * FB:(hc + 1) * FB], ops[:])
                nc.sync.dma_start(
                    po_h[:].rearrange("(tt p) h -> p tt h", p=P)[:, tt, :],
                    po_t[:, tt, :],
                )

    nc.finalize()
    return nc


def _build_expert(C):
    """Launch B: one expert FFN over C routed tokens, compensated fp8."""
    import concourse.bacc as bacc
    import concourse.mybir as mybir
    import concourse.tile as tile

    F32, F8 = mybir.dt.float32, mybir.dt.float8e4
    AF = mybir.ActivationFunctionType
    DR = mybir.MatmulPerfMode.DoubleRow

    assert C % P == 0 and C >= 512, C
    NCH = max(1, (C + 511) // 512)
    assert C % NCH == 0, (C, NCH)
    CN = C // NCH
    assert 256 <= CN <= 512, CN
    CT = C // P

    nc = bacc.Bacc()
    xm_h = nc.dram_tensor("xm", [H, C], F8, kind="ExternalInput")   # SX*x main
    xr_h = nc.dram_tensor("xr", [H, C], F8, kind="ExternalInput")   # SX*x residual
    w1m_h = nc.dram_tensor("w1m", [H, FF], F8, kind="ExternalInput")
    w1r_h = nc.dram_tensor("w1r", [H, FF], F8, kind="ExternalInput")
    w2m_h = nc.dram_tensor("w2m", [FF, H], F8, kind="ExternalInput")
    w2r_h = nc.dram_tensor("w2r", [FF, H], F8, kind="ExternalInput")
    b1_h = nc.dram_tensor("b1c", [P, MF], F32, kind="ExternalInput")
    gs_h = nc.dram_tensor("gsc", [P, CT], F32, kind="ExternalInput")  # gate/SW per token
    y_h = nc.dram_tensor("y", [C, H], F32, kind="ExternalOutput")

    with tile.TileContext(nc) as tc:
        with tc.tile_pool(name="consts", bufs=1) as consts, \
             tc.tile_pool(name="big", bufs=1) as big, \
             tc.tile_pool(name="w1s", bufs=3) as w1s, \
             tc.tile_pool(name="ht", bufs=2) as htp, \
             tc.tile_pool(name="ps_mm", bufs=3, space="PSUM") as ps_mm:

            b1_t = consts.tile([P, MF], F32)
            nc.sync.dma_start(b1_t[:], b1_h[:])
            gs_t = consts.tile([P, CT], F32)
            nc.sync.dma_start(gs_t[:], gs_h[:])

            # x and W2-main on the Act DMA queue (idle early); W2-residual
            # interleaved into the SP W1-chunk stream (slack-scheduled)
            xm_t = big.tile([P, KO, C], F8)
            nc.scalar.dma_start(xm_t[:], xm_h[:].rearrange("(ko p) c -> p ko c", p=P))
            xr_t = big.tile([P, KO, C], F8)
            nc.scalar.dma_start(xr_t[:], xr_h[:].rearrange("(ko p) c -> p ko c", p=P))
            w2m_t = big.tile([P, MF, H], F8)
            w2r_t = big.tile([P, MF, H], F8)

            hm_t = big.tile([P, MF, C], F8)
            hr_t = big.tile([P, MF, C], F8)

            # ---- W1 pass: h = gelu((xm+xr)@(w1m+w1r)/(SX*SW) + b1) ----
            for mfc in range(FF // FB):            # 8 chunks of 512 cols
                w1m_c = w1s.tile([P, KO, FB], F8, tag="w1m")
                nc.sync.dma_start(
                    w1m_c[:],
                    w1m_h[:, mfc * FB:(mfc + 1) * FB].rearrange("(ko p) f -> p ko f", p=P),
                )
                w1r_c = w1s.tile([P, KO, FB], F8, tag="w1r")
                nc.sync.dma_start(
                    w1r_c[:],
                    w1r_h[:, mfc * FB:(mfc + 1) * FB].rearrange("(ko p) f -> p ko f", p=P),
                )
                if mfc == 3:
                    nc.sync.dma_start(
                        w2m_t[:], w2m_h[:].rearrange("(fc p) h -> p fc h", p=P))
                elif mfc == 5:
                    nc.sync.dma_start(
                        w2r_t[:], w2r_h[:].rearrange("(fc p) h -> p fc h", p=P))
                for mf in range(4):
                    mfg = mfc * 4 + mf
                    for nch in range(NCH):
                        cs = slice(nch * CN, (nch + 1) * CN)
                        hps = ps_mm.tile([P, 512], F32, tag="mm")
                        terms = [(w1m_c, xm_t), (w1m_c, xr_t), (w1r_c, xm_t)]
                        nterm = len(terms)
                        for ti, (wt, xt) in enumerate(terms):
                            for kc2 in range(KO // 2):
                                nc.tensor.matmul(
                                    hps[:, :CN],
                                    wt[:, 2 * kc2:2 * kc2 + 2, mf * P:(mf + 1) * P],
                                    xt[:, 2 * kc2:2 * kc2 + 2, cs],
                                    start=(ti == 0 and kc2 == 0),
                                    stop=(ti == nterm - 1 and kc2 == KO // 2 - 1),
                                    perf_mode=DR,
                                )
                        ht = htp.tile([P, 512], F32, tag="ht")
                        nc.scalar.activation(
                            ht[:, :CN], hps[:, :CN], AF.Gelu_apprx_tanh,
                            bias=b1_t[:, mfg:mfg + 1], scale=1.0 / (SX * SW),
                        )
                        # split the fp8 quantize-copy across Act/DVE
                        if mfg % 2 == 0:
                            nc.scalar.activation(hm_t[:, mfg, cs], ht[:, :CN], AF.Copy)
                        else:
                            nc.vector.tensor_copy(hm_t[:, mfg, cs], ht[:, :CN])
                        nc.vector.tensor_sub(hr_t[:, mfg, cs], ht[:, :CN], hm_t[:, mfg, cs])

            # ---- W2 pass: y = (h@(w2m+w2r))/SW * gate, token-major ----
            y_t = big.tile([P, CT, H], F32)
            for ct in range(CT):
                for hc in range(2):
                    yps = ps_mm.tile([P, 512], F32, tag="mm")
                    terms = [(hm_t, w2m_t), (hr_t, w2m_t), (hm_t, w2r_t)]
                    nterm = len(terms)
                    for ti, (ht_, wt) in enumerate(terms):
                        for fc2 in range(MF // 2):
                            nc.tensor.matmul(
                                yps[:],
                                ht_[:, 2 * fc2:2 * fc2 + 2, ct * P:(ct + 1) * P],
                                wt[:, 2 * fc2:2 * fc2 + 2, hc * FB:(hc + 1) * FB],
                                start=(ti == 0 and fc2 == 0),
                                stop=(ti == nterm - 1 and fc2 == MF // 2 - 1),
                                perf_mode=DR,
                            )
                    nc.scalar.activation(
                        y_t[:, ct, hc * FB:(hc + 1) * FB], yps[:],
                        AF.Copy, scale=gs_t[:, ct:ct + 1],
                    )
                nc.sync.dma_start(
                    y_h[:].rearrange("(ct p) h -> p ct h", p=P)[:, ct, :],
                    y_t[:, ct, :],
                )

    nc.finalize()
    return nc


def _get_attn():
    if "attn" not in _CACHE:
        _CACHE["attn"] = _build_attn()
    return _CACHE["attn"]


def _get_expert(C):
    key = ("exp", C)
    if key not in _CACHE:
        _CACHE[key] = _build_expert(C)
    return _CACHE[key]


def _ln(x):
    m = x.mean(-1, keepdims=True)
    v = x.var(-1, keepdims=True)
    return (x - m) / np.sqrt(v + EPS)


def _q8(a, s):
    """e4m3 quantize a*s, return (main, residual) as raw scaled fp8 arrays."""
    import ml_dtypes
    e4 = ml_dtypes.float8_e4m3
    m = (a * s).astype(np.float32).astype(e4)
    r = ((a * s).astype(np.float32) - m.astype(np.float32)).astype(e4)
    return m, r


def _colt(vec):
    v32 = np.ascontiguousarray(np.asarray(vec, dtype=np.float32))
    return np.ascontiguousarray(v32.reshape(-1, P).T)


def kernel(**inputs):
    import ml_dtypes
    from concourse.bass_utils import run_bass_kernel_spmd

    bf16 = ml_dtypes.bfloat16
    f = lambda k: np.asarray(inputs[k], dtype=np.float32)
    x = f("hidden_states")                        # [B, S, H]
    mask = np.asarray(inputs["attention_mask"])   # [B, S] int32
    ln1_g, ln1_b = f("ln1_g").astype(np.float64), f("ln1_b").astype(np.float64)
    ln2_g, ln2_b = f("ln2_g").astype(np.float64), f("ln2_b").astype(np.float64)
    Wq, Wk, Wv, Wo = (f(k).astype(np.float64) for k in ("Wq", "Wk", "Wv", "Wo"))
    bq, bk, bv, bo = (f(k).astype(np.float64) for k in ("bq", "bk", "bv", "bo"))
    level_logits = f("level_logits").astype(np.float64)
    Wr, br = f("Wr").astype(np.float64), f("br").astype(np.float64)
    W1, b1 = f("W1").astype(np.float64), f("b1").astype(np.float64)
    W2, b2 = f("W2").astype(np.float64), f("b2").astype(np.float64)

    # ---- host folding (as in reference, with LN1 gamma/beta absorbed) ----
    scale = 1.0 / np.sqrt(DH)
    wq_eff = (ln1_g[None, :, None] * Wq) * scale               # [L,H,H]
    bq_eff = (bq + ln1_b @ Wq) * scale                         # [L,H]
    wk_eff = ln1_g[None, :, None] * Wk
    bk_eff = bk + ln1_b @ Wk
    wv_eff = ln1_g[None, :, None] * Wv
    bv_eff = bv + ln1_b @ Wv
    lw = np.exp(level_logits - level_logits.max())
    lw = lw / lw.sum()
    wo_eff = lw[:, None, None] * Wo
    boc_eff = np.einsum("l,lh->h", lw, bo) + np.einsum("lf,lfh->h", bv_eff, wo_eff)
    wo_dev = wo_eff / SV          # device ctx carries a factor of SV

    xn1 = _ln(x.astype(np.float64))                            # LN1 sans gamma/beta
    xn1_T32 = np.ascontiguousarray(
        np.swapaxes(xn1.astype(np.float32), 1, 2))             # [B,H,S]
    mbias = (1.0 - mask.astype(np.float32)) * np.float32(-1e9)
    mb_dev = mbias + np.float32(np.log(SP))                    # exp out pre-scaled by SP

    xq8 = [_q8(xn1_T32[b], SXA) for b in range(B)]
    w8 = {}
    for l in range(L):
        for hb in range(2):
            fs = slice(hb * FB, (hb + 1) * FB)
            w8[(l, hb)] = (
                _q8(np.ascontiguousarray(wq_eff[l][:, fs].astype(np.float32)), SWQ),
                _q8(np.ascontiguousarray(wk_eff[l][:, fs].astype(np.float32)), SWK),
                _q8(np.ascontiguousarray(wv_eff[l][:, fs].astype(np.float32)), SWV),
            )
    in_maps = []
    for c in range(NCORES):
        b, g = c // 4, c % 4
        l, hb = g // 2, g % 2
        fs = slice(hb * FB, (hb + 1) * FB)
        (wqm, wqr), (wkm, wkr), (wvm, wvr) = w8[(l, hb)]
        in_maps.append({
            "xam": xq8[b][0], "xar": xq8[b][1],
            "wqm": wqm, "wqr": wqr,
            "wkm": wkm, "wkr": wkr,
            "wvm": wvm, "wvr": wvr,
            "wo": np.ascontiguousarray(wo_dev[l][fs, :].astype(np.float32)).astype(bf16),
            "bqc": _colt(bq_eff[l][fs]),
            "bkc": _colt(bk_eff[l][fs]),
            "mb": _colt(mb_dev[b]),
        })

    import time as _time
    nc_a = _get_attn()
    t0 = _time.time()
    res_a = run_bass_kernel_spmd(nc_a, in_maps, core_ids=list(range(NCORES)))
    _PERF["a_wall_s"] = _time.time() - t0

    # device xres = x + sum of partials + boc
    xres = x.reshape(-1, H).copy()
    for c in range(NCORES):
        b = c // 4
        xres[b * S:(b + 1) * S] += res_a.results[c]["po"].astype(np.float32)
    xres += boc_eff.astype(np.float32)[None, :]

    # ---- host: exact fp32 attention -> routing (LN2, router, top-2) ----
    t0 = _time.time()
    xn1h = (xn1 * ln1_g + ln1_b).astype(np.float32).reshape(-1, H)
    attn_h = np.zeros((B * S, H), np.float32)
    lw32 = lw.astype(np.float32)
    for l in range(L):
        q = (xn1h @ Wq[l].astype(np.float32) + bq[l].astype(np.float32)) \
            .reshape(B, S, NH, DH) * np.float32(scale)
        k = (xn1h @ Wk[l].astype(np.float32) + bk[l].astype(np.float32)) \
            .reshape(B, S, NH, DH)
        v = (xn1h @ Wv[l].astype(np.float32) + bv[l].astype(np.float32)) \
            .reshape(B, S, NH, DH)
        ol = np.empty((B, S, H), np.float32)
        for b_ in range(B):
            mrow = mbias[b_][None, :]
            for n in range(NH):
                sc = q[b_, :, n] @ k[b_, :, n].T + mrow
                sc -= sc.max(-1, keepdims=True)
                e = np.exp(sc)
                a = e / e.sum(-1, keepdims=True)
                ol[b_, :, n * DH:(n + 1) * DH] = a @ v[b_, :, n]
        attn_h += lw32[l] * (
            ol.reshape(-1, H) @ Wo[l].astype(np.float32) + bo[l].astype(np.float32))
    xres_h = x.reshape(-1, H) + attn_h
    xn2 = (_ln(xres_h.astype(np.float64)) * ln2_g + ln2_b)     # [T,H] fp64
    logits = xn2 @ Wr + br
    pm = logits.max(-1, keepdims=True)
    probs = np.exp(logits - pm)
    probs /= probs.sum(-1, keepdims=True)
    order = np.argsort(-probs, axis=-1, kind="stable")
    topi = order[:, :TOPK]
    topv = np.take_along_axis(probs, topi, axis=-1)
    gates = topv / topv.sum(-1, keepdims=True)                 # [T,2]
    _PERF["host_route_s"] = _time.time() - t0

    tok_idx, gate_val = [], []
    for e in range(E):
        sel = np.nonzero(topi == e)
        tok_idx.append(sel[0])
        gate_val.append(gates[sel[0], sel[1]])
    counts = [len(t) for t in tok_idx]
    C = max(512, ((max(counts) + P - 1) // P) * P)
    while True:
        nch = (C + 511) // 512
        if C % nch == 0 and C // nch >= 256:
            break
        C += P

    w1f = ln2_g[None, :, None] * W1                            # [E,H,F]
    b1f = b1 + ln2_b @ W1                                      # [E,F]
    xn2_32 = xn2.astype(np.float32)
    e4 = ml_dtypes.float8_e4m3

    in_maps_b = []
    for e in range(E):
        xe = np.zeros((C, H), np.float32)
        xe[:counts[e]] = xn2_32[tok_idx[e]]
        xmq, xrq = _q8(np.ascontiguousarray(xe.T), SX)         # [H,C] fp8
        w1mq, w1rq = _q8(w1f[e], SW)
        w2mq, w2rq = _q8(W2[e], SW)
        g = np.zeros((C,), np.float32)
        g[:counts[e]] = gate_val[e].astype(np.float32) / np.float32(SW)
        in_maps_b.append({
            "xm": xmq, "xr": xrq,
            "w1m": np.ascontiguousarray(w1mq.astype(e4)),
            "w1r": np.ascontiguousarray(w1rq.astype(e4)),
            "w2m": np.ascontiguousarray(w2mq.astype(e4)),
            "w2r": np.ascontiguousarray(w2rq.astype(e4)),
            "b1c": _colt(b1f[e]),
            "gsc": np.ascontiguousarray(g.reshape(-1, P).T),
            "y": None,
        })
    for m in in_maps_b:
        del m["y"]

    nc_b = _get_expert(C)
    t0 = _time.time()
    res_b = run_bass_kernel_spmd(nc_b, in_maps_b, core_ids=list(range(NCORES)))
    _PERF["b_wall_s"] = _time.time() - t0
    _PERF["capacity"] = C
    _PERF["counts"] = counts

    out = xres
    for e in range(E):
        if counts[e]:
            out[tok_idx[e]] += res_b.results[e]["y"][:counts[e]]
    # b2 contribution: sum_e gate_e * b2[e]
    gmat = np.zeros((B * S, E), np.float64)
    np.put_along_axis(gmat, topi, gates, axis=-1)
    out += (gmat @ b2).astype(np.float32)
    return out.reshape(B, S, H).astype(np.float32)


# revision 45
# speedup vs baseline: 1.2296x; 1.0356x over previous
"""Trainium2 Bass kernel for nn_MoEMLABlock (MoE + multi-level attention block).

Strategy (8 NeuronCores, full inputs in / full output out):
  Launch A (head-parallel attention): 64 attention instances (level l,
    batch b, head h) are split as: core c -> batch b=c//4, level l=(c%4)//2,
    head block hb=(c%4)%2 (8 heads / 512 feature cols).  Each projection is
    computed exactly once across cores (no K/V recompute).  LayerNorm 1,
    the 1/sqrt(DH) scale and the level-softmax weight are folded into the
    projection weights on the host; V carries no bias (its effect, plus bo,
    is the constant `boc` added on the host).  Device math is bf16 inputs
    with fp32 PSUM accumulation.  Each core emits its partial attention
    output [S, H] token-major; host sums partials + residual.
  Host: exact fp32 attention recompute (BLAS) for ROUTING ONLY -- router
    logits, top-2 and gates are bit-robust against device rounding (the
    tightest p2/p3 logit gap in this data is 3e-5, so routing must not
    depend on device numerics).  Also LN2 + expert input prep.
  Launch B (expert-parallel FFN): core e runs expert e over its routed
    tokens with error-compensated fp8 (e4m3) DoubleRow matmuls:
    x@W1 ~= xm@W1m + xr@W1m + xm@W1r where *m/*r are fp8 main/residual
    parts (measured output error 5e-4 rel, 4x PE throughput per term).
    Gates are applied on device via the activation scale; the b2 bias is
    applied on the host (gates @ b2).
  Host: scatter-add combine + residuals.
"""

import numpy as np

H = 1024
NH = 16
DH = 64
L = 2
E = 8
FF = 4096
B = 2
S = 1024
EPS = 1e-5
P = 128
NCORES = 8
KO = H // P            # 8 contraction chunks over H
TT = S // P            # 8 token tiles
FB = 512               # feature block per core (8 heads)
MF = FF // P           # 32
TOPK = 2

SX = 16.0              # fp8 scale for expert input x
SW = 64.0              # fp8 scale for expert weights
SP = 4.0               # fp8 scale for attention exp-scores
SV = 16.0              # fp8 scale for attention V
SXA = 16.0             # fp8 scale for attention input LN1(x)
SWQ = 512.0            # fp8 scale for wq (carries the 1/sqrt(DH) fold)
SWK = 64.0             # fp8 scale for wk
SWV = 64.0             # fp8 scale for wv

_CACHE = {}
_PERF = {}


def _build_attn():
    """Launch A: one (batch, level, 8-head block) per core."""
    import concourse.bacc as bacc
    import concourse.mybir as mybir
    import concourse.tile as tile

    F32, F32R, BF16 = mybir.dt.float32, mybir.dt.float32r, mybir.dt.bfloat16
    F8 = mybir.dt.float8e4
    AF = mybir.ActivationFunctionType
    DR = mybir.MatmulPerfMode.DoubleRow

    nc = bacc.Bacc()
    xam_h = nc.dram_tensor("xam", [H, S], F8, kind="ExternalInput")     # SXA*LN1(x_b)^T main
    xar_h = nc.dram_tensor("xar", [H, S], F8, kind="ExternalInput")     # ... residual
    wqm_h = nc.dram_tensor("wqm", [H, FB], F8, kind="ExternalInput")
    wqr_h = nc.dram_tensor("wqr", [H, FB], F8, kind="ExternalInput")
    wkm_h = nc.dram_tensor("wkm", [H, FB], F8, kind="ExternalInput")
    wkr_h = nc.dram_tensor("wkr", [H, FB], F8, kind="ExternalInput")
    wvm_h = nc.dram_tensor("wvm", [H, FB], F8, kind="ExternalInput")
    wvr_h = nc.dram_tensor("wvr", [H, FB], F8, kind="ExternalInput")
    wo_h = nc.dram_tensor("wo", [FB, H], BF16, kind="ExternalInput")
    bq_h = nc.dram_tensor("bqc", [P, 4], F32, kind="ExternalInput")
    bk_h = nc.dram_tensor("bkc", [P, 4], F32, kind="ExternalInput")
    mb_h = nc.dram_tensor("mb", [P, TT], F32, kind="ExternalInput")     # key mask bias + ln(SP)
    po_h = nc.dram_tensor("po", [S, H], BF16, kind="ExternalOutput")    # partial attn out

    with tile.TileContext(nc) as tc:
        with tc.tile_pool(name="consts", bufs=1) as consts, \
             tc.tile_pool(name="big", bufs=1) as big, \
             tc.tile_pool(name="work", bufs=5) as work, \
             tc.tile_pool(name="ps_mm", bufs=2, space="PSUM") as ps_mm, \
             tc.tile_pool(name="ps_sc", bufs=2, space="PSUM") as ps_sc, \
             tc.tile_pool(name="ps_cx", bufs=2, space="PSUM") as ps_cx:

            ones_f = consts.tile([P, 1], F32)
            nc.vector.memset(ones_f[:], 1.0)
            ones_row = consts.tile([1, P], F32R)
            nc.vector.tensor_copy(ones_row[:], ones_f[:1, :].to_broadcast((1, P)))

            bq_t = consts.tile([P, 4], F32)
            nc.sync.dma_start(bq_t[:], bq_h[:])
            bk_t = consts.tile([P, 4], F32)
            nc.sync.dma_start(bk_t[:], bk_h[:])
            mb_t = consts.tile([P, TT], F32)
            nc.sync.dma_start(mb_t[:], mb_h[:])

            # Two DMA queues (SP + Act): Q/K path inputs on SP in first-use
            # order; V weights + wo in parallel on the Act queue.
            def ld(dst, src, eng):
                eng.dma_start(dst[:], src.rearrange("(ko p) f -> p ko f", p=P))

            wqm_t = big.tile([P, KO, FB], F8)
            ld(wqm_t, wqm_h[:], nc.sync)
            wqr_t = big.tile([P, KO, FB], F8)
            ld(wqr_t, wqr_h[:], nc.sync)
            xm_a = big.tile([P, KO, FB], F8)       # tokens 0..511
            ld(xm_a, xam_h[:, :FB], nc.sync)
            xr_a = big.tile([P, KO, FB], F8)
            ld(xr_a, xar_h[:, :FB], nc.sync)
            xm_b = big.tile([P, KO, FB], F8)       # tokens 512..1023
            ld(xm_b, xam_h[:, FB:], nc.sync)
            xr_b = big.tile([P, KO, FB], F8)
            ld(xr_b, xar_h[:, FB:], nc.sync)
            wkm_t = big.tile([P, KO, FB], F8)
            ld(wkm_t, wkm_h[:], nc.sync)
            wkr_t = big.tile([P, KO, FB], F8)
            ld(wkr_t, wkr_h[:], nc.sync)
            wvm_t = big.tile([P, KO, FB], F8)
            ld(wvm_t, wvm_h[:], nc.scalar)
            wvr_t = big.tile([P, KO, FB], F8)
            ld(wvr_t, wvr_h[:], nc.scalar)
            wo_t = big.tile([P, 4, H], BF16)
            nc.scalar.dma_start(wo_t[:], wo_h[:].rearrange("(fc p) h -> p fc h", p=P))

            def xtok(src, t0):
                # [P, KO, 128] token block starting at t0 (main or residual)
                return src[:, :, t0 % FB:t0 % FB + P]

            # ---- tiles; emission order below keeps the Act exp chain (the
            #      pacer) fed from early on while PE fills with proj work ----
            v_t = big.tile([P, TT, 8, DH + 2], F8)
            nc.vector.memset(v_t[:, :, :, DH:DH + 1], 1.0)
            nc.vector.memset(v_t[:, :, :, DH + 1:DH + 2], 0.0)
            q_t = big.tile([P, 4, S], BF16)
            k_t = big.tile([P, 4, S], BF16)
            ctx = big.tile([P, 4, S], BF16)
            p_sbs = {}

            def comp_mm(out_ap, x_of, w_of, x_is_lhs):
                """3-term compensated fp8 DoubleRow accumulation group:
                xm@wm + xr@wm + xm@wr."""
                terms = [(0, 0), (1, 0), (0, 1)]   # (x residual?, w residual?)
                for ti, (xi, wi) in enumerate(terms):
                    for kc2 in range(KO // 2):
                        ks = slice(2 * kc2, 2 * kc2 + 2)
                        xa, wa = x_of(xi, ks), w_of(wi, ks)
                        lhsT, rhs = (xa, wa) if x_is_lhs else (wa, xa)
                        nc.tensor.matmul(
                            out_ap, lhsT, rhs,
                            start=(ti == 0 and kc2 == 0),
                            stop=(ti == 2 and kc2 == KO // 2 - 1),
                            perf_mode=DR,
                        )

            def emit_v(half):
                for tt in range(half * 4, half * 4 + 4):
                    xmx = xm_a if tt < 4 else xm_b
                    xrx = xr_a if tt < 4 else xr_b
                    vps = ps_mm.tile([P, FB], F32, tag="mm")
                    comp_mm(
                        vps[:],
                        lambda xi, ks: xtok(xrx if xi else xmx, tt * P)[:, ks, :],
                        lambda wi, ks: (wvr_t if wi else wvm_t)[:, ks, :],
                        x_is_lhs=True,
                    )
                    nc.vector.tensor_scalar_mul(
                        v_t[:, tt, :, 0:DH],
                        vps[:].rearrange("p (h d) -> p h d", d=DH),
                        float(SV / (SXA * SWV)),
                    )

            def emit_qk(ft):
                for dst, wm, wr, bsrc, sw in (
                        (q_t, wqm_t, wqr_t, bq_t, SWQ),
                        (k_t, wkm_t, wkr_t, bk_t, SWK)):
                    for qc in range(2):
                        xmx = xm_a if qc == 0 else xm_b
                        xrx = xr_a if qc == 0 else xr_b
                        pps = ps_mm.tile([P, FB], F32, tag="mm")
                        comp_mm(
                            pps[:],
                            lambda xi, ks: (xrx if xi else xmx)[:, ks, :],
                            lambda wi, ks: (wr if wi else wm)[:, ks, ft * P:(ft + 1) * P],
                            x_is_lhs=False,
                        )
                        nc.vector.tensor_scalar(
                            dst[:, ft, qc * FB:(qc + 1) * FB], pps[:],
                            float(1.0 / (SXA * sw)), bsrc[:, ft:ft + 1],
                            op0=mybir.AluOpType.mult, op1=mybir.AluOpType.add,
                        )

            def emit_sc(h):
                ft, pb = h // 2, (h % 2) * DH
                p_sb = work.tile([P, TT, S], F8, tag="p_sb")
                p_sbs[h] = p_sb
                for tt in range(TT):
                    sps = ps_sc.tile([P, S], F32, tag="sc")
                    for qc in range(2):
                        nc.tensor.matmul(
                            sps[:, qc * FB:(qc + 1) * FB],
                            k_t[pb:pb + DH, ft, tt * P:(tt + 1) * P],
                            q_t[pb:pb + DH, ft, qc * FB:(qc + 1) * FB],
                            start=True, stop=True,
                        )
                    nc.scalar.activation(
                        p_sb[:, tt, :], sps[:],
                        AF.Exp, bias=mb_t[:, tt:tt + 1],
                    )

            def emit_ctx(h):
                ft, pb = h // 2, (h % 2) * DH
                p_sb = p_sbs[h]
                for qc in range(2):
                    cxps = ps_cx.tile([DH + 2, FB], F32, tag="cx")
                    for t2 in range(TT // 2):
                        nc.tensor.matmul(
                            cxps[:],
                            v_t[:, 2 * t2:2 * t2 + 2, h, :],
                            p_sb[:, 2 * t2:2 * t2 + 2, qc * FB:(qc + 1) * FB],
                            start=(t2 == 0), stop=(t2 == TT // 2 - 1),
                            perf_mode=DR,
                        )
                    r = work.tile([1, FB], F32R, tag="r")
                    with nc.allow_low_precision(reason="softmax recip feeds broadcast matmul"):
                        nc.vector.reciprocal(r[:], cxps[DH:DH + 1, :])
                    rbps = ps_mm.tile([P, FB], F32, tag="mm")
                    nc.tensor.matmul(rbps[:DH, :], ones_row[:, :DH], r[:], start=True, stop=True)
                    rb_sb = work.tile([DH, FB], F32, tag="rb_sb")
                    nc.vector.tensor_copy(rb_sb[:], rbps[:DH, :])
                    nc.vector.tensor_mul(
                        ctx[pb:pb + DH, ft, qc * FB:(qc + 1) * FB],
                        cxps[0:DH, :], rb_sb[:],
                    )

            emit_qk(0)
            emit_sc(0)
            emit_sc(1)
            emit_v(0)
            emit_qk(1)
            emit_sc(2)
            emit_v(1)
            emit_ctx(0)
            emit_sc(3)
            emit_ctx(1)
            emit_qk(2)
            emit_sc(4)
            emit_sc(5)
            emit_ctx(2)
            emit_ctx(3)
            emit_qk(3)
            emit_sc(6)
            emit_sc(7)
            emit_ctx(4)
            emit_ctx(5)
            emit_ctx(6)
            emit_ctx(7)

            # ---- O-projection straight to token-major, DMA out ----
            po_t = big.tile([P, TT, H], BF16)
            for tt in range(TT):
                for hc in range(2):
                    ops = ps_mm.tile([P, FB], F32, tag="mm")
                    for fc in range(4):
                        nc.tensor.matmul(
                            ops[:],
                            ctx[:, fc, tt * P:(tt + 1) * P],
                            wo_t[:, fc, hc * FB:(hc + 1) * FB],
                            start=(fc == 0), stop=(fc == 3),
                        )
                    nc.vector.tensor_copy(po_t[:, tt, hc * FB:(hc + 1) * FB], ops[:])
                nc.sync.dma_start(
                    po_h[:].rearrange("(tt p) h -> p tt h", p=P)[:, tt, :],
                    po_t[:, tt, :],
                )

    nc.finalize()
    return nc


def _build_expert(C):
    """Launch B: one expert FFN over C routed tokens, compensated fp8.

    Feature-major throughout: both passes cost ~C columns (no 128-token
    tile rounding), so C only needs 32-alignment.  Output y is [H, C];
    the host transposes.  Gates (pre-divided by SW) are broadcast to all
    partitions via a ones-matmul and applied on the DVE.
    """
    import concourse.bacc as bacc
    import concourse.mybir as mybir
    import concourse.tile as tile

    F32, F32R, F8 = mybir.dt.float32, mybir.dt.float32r, mybir.dt.float8e4
    AF = mybir.ActivationFunctionType
    DR = mybir.MatmulPerfMode.DoubleRow

    assert C % 32 == 0 and C >= 512, C
    NCH = max(1, (C + 511) // 512)
    assert C % NCH == 0, (C, NCH)
    CN = C // NCH
    assert 256 <= CN <= 512, CN

    nc = bacc.Bacc()
    xm_h = nc.dram_tensor("xm", [H, C], F8, kind="ExternalInput")   # SX*x main
    xr_h = nc.dram_tensor("xr", [H, C], F8, kind="ExternalInput")   # SX*x residual
    w1m_h = nc.dram_tensor("w1m", [H, FF], F8, kind="ExternalInput")
    w1r_h = nc.dram_tensor("w1r", [H, FF], F8, kind="ExternalInput")
    w2m_h = nc.dram_tensor("w2m", [FF, H], F8, kind="ExternalInput")
    w2r_h = nc.dram_tensor("w2r", [FF, H], F8, kind="ExternalInput")
    b1_h = nc.dram_tensor("b1c", [P, MF], F32, kind="ExternalInput")
    g_h = nc.dram_tensor("gsr", [1, C], F32, kind="ExternalInput")  # gate/SW per token
    y_h = nc.dram_tensor("y", [H, C], F32, kind="ExternalOutput")   # feature-major

    with tile.TileContext(nc) as tc:
        with tc.tile_pool(name="consts", bufs=1) as consts, \
             tc.tile_pool(name="big", bufs=1) as big, \
             tc.tile_pool(name="w1s", bufs=3) as w1s, \
             tc.tile_pool(name="ht", bufs=2) as htp, \
             tc.tile_pool(name="ps_mm", bufs=3, space="PSUM") as ps_mm:

            b1_t = consts.tile([P, MF], F32)
            nc.sync.dma_start(b1_t[:], b1_h[:])
            ones_f = consts.tile([P, 1], F32)
            nc.vector.memset(ones_f[:], 1.0)
            ones_row = consts.tile([1, P], F32R)
            nc.vector.tensor_copy(ones_row[:], ones_f[:1, :].to_broadcast((1, P)))
            g_sb = consts.tile([1, C], F32R)
            nc.sync.dma_start(g_sb[:], g_h[:].bitcast(F32R))

            # x on the Act DMA queue (idle early); W2 interleaved into the
            # SP W1-chunk stream (slack-scheduled)
            xm_t = big.tile([P, KO, C], F8)
            nc.scalar.dma_start(xm_t[:], xm_h[:].rearrange("(ko p) c -> p ko c", p=P))
            xr_t = big.tile([P, KO, C], F8)
            nc.scalar.dma_start(xr_t[:], xr_h[:].rearrange("(ko p) c -> p ko c", p=P))
            w2m_t = big.tile([P, MF, H], F8)
            w2r_t = big.tile([P, MF, H], F8)

            hm_t = big.tile([P, MF, C], F8)
            hr_t = big.tile([P, MF, C], F8)

            # gate row broadcast to all partitions (gb[p, c] = gate_c / SW)
            gb_sb = big.tile([P, C], F32)
            for nch in range(NCH):
                cs = slice(nch * CN, (nch + 1) * CN)
                gps = ps_mm.tile([P, 512], F32, tag="mm")
                nc.tensor.matmul(gps[:, :CN], ones_row[:], g_sb[:, cs],
                                 start=True, stop=True)
                nc.vector.tensor_copy(gb_sb[:, cs], gps[:, :CN])

            # ---- W1 pass: h = gelu((xm+xr)@(w1m+w1r)/(SX*SW) + b1) ----
            for mfc in range(FF // FB):            # 8 chunks of 512 cols
                w1m_c = w1s.tile([P, KO, FB], F8, tag="w1m")
                nc.sync.dma_start(
                    w1m_c[:],
                    w1m_h[:, mfc * FB:(mfc + 1) * FB].rearrange("(ko p) f -> p ko f", p=P),
                )
                w1r_c = w1s.tile([P, KO, FB], F8, tag="w1r")
                nc.sync.dma_start(
                    w1r_c[:],
                    w1r_h[:, mfc * FB:(mfc + 1) * FB].rearrange("(ko p) f -> p ko f", p=P),
                )
                if mfc == 3:
                    nc.sync.dma_start(
                        w2m_t[:], w2m_h[:].rearrange("(fc p) h -> p fc h", p=P))
                elif mfc == 5:
                    nc.sync.dma_start(
                        w2r_t[:], w2r_h[:].rearrange("(fc p) h -> p fc h", p=P))
                for mf in range(4):
                    mfg = mfc * 4 + mf
                    for nch in range(NCH):
                        cs = slice(nch * CN, (nch + 1) * CN)
                        hps = ps_mm.tile([P, 512], F32, tag="mm")
                        terms = [(w1m_c, xm_t), (w1m_c, xr_t), (w1r_c, xm_t)]
                        nterm = len(terms)
                        for ti, (wt, xt) in enumerate(terms):
                            for kc2 in range(KO // 2):
                                nc.tensor.matmul(
                                    hps[:, :CN],
                                    wt[:, 2 * kc2:2 * kc2 + 2, mf * P:(mf + 1) * P],
                                    xt[:, 2 * kc2:2 * kc2 + 2, cs],
                                    start=(ti == 0 and kc2 == 0),
                                    stop=(ti == nterm - 1 and kc2 == KO // 2 - 1),
                                    perf_mode=DR,
                                )
                        ht = htp.tile([P, 512], F32, tag="ht")
                        nc.scalar.activation(
                            ht[:, :CN], hps[:, :CN], AF.Gelu_apprx_tanh,
                            bias=b1_t[:, mfg:mfg + 1], scale=1.0 / (SX * SW),
                        )
                        # split the fp8 quantize-copy across Act/DVE
                        if mfg % 2 == 0:
                            nc.scalar.activation(hm_t[:, mfg, cs], ht[:, :CN], AF.Copy)
                        else:
                            nc.vector.tensor_copy(hm_t[:, mfg, cs], ht[:, :CN])
                        nc.vector.tensor_sub(hr_t[:, mfg, cs], ht[:, :CN], hm_t[:, mfg, cs])

            # ---- W2 pass, feature-major: y[h, c] = (h@W2)[h, c] * gb[c] ----
            y_t = big.tile([P, KO, C], F32)
            for hc in range(KO):
                for nch in range(NCH):
                    cs = slice(nch * CN, (nch + 1) * CN)
                    yps = ps_mm.tile([P, 512], F32, tag="mm")
                    terms = [(hm_t, w2m_t), (hr_t, w2m_t), (hm_t, w2r_t)]
                    nterm = len(terms)
                    for ti, (ht_, wt) in enumerate(terms):
                        for fc2 in range(MF // 2):
                            nc.tensor.matmul(
                                yps[:, :CN],
                                wt[:, 2 * fc2:2 * fc2 + 2, hc * P:(hc + 1) * P],
                                ht_[:, 2 * fc2:2 * fc2 + 2, cs],
                                start=(ti == 0 and fc2 == 0),
                                stop=(ti == nterm - 1 and fc2 == MF // 2 - 1),
                                perf_mode=DR,
                            )
                    nc.vector.tensor_mul(
                        y_t[:, hc, cs], yps[:, :CN], gb_sb[:, cs],
                    )
                nc.sync.dma_start(
                    y_h[:].rearrange("(hc p) c -> p hc c", p=P)[:, hc, :],
                    y_t[:, hc, :],
                )

    nc.finalize()
    return nc


def _get_attn():
    if "attn" not in _CACHE:
        _CACHE["attn"] = _build_attn()
    return _CACHE["attn"]


def _get_expert(C):
    key = ("exp", C)
    if key not in _CACHE:
        _CACHE[key] = _build_expert(C)
    return _CACHE[key]


def _ln(x):
    m = x.mean(-1, keepdims=True)
    v = x.var(-1, keepdims=True)
    return (x - m) / np.sqrt(v + EPS)


def _q8(a, s):
    """e4m3 quantize a*s, return (main, residual) as raw scaled fp8 arrays."""
    import ml_dtypes
    e4 = ml_dtypes.float8_e4m3
    m = (a * s).astype(np.float32).astype(e4)
    r = ((a * s).astype(np.float32) - m.astype(np.float32)).astype(e4)
    return m, r


def _colt(vec):
    v32 = np.ascontiguousarray(np.asarray(vec, dtype=np.float32))
    return np.ascontiguousarray(v32.reshape(-1, P).T)


def kernel(**inputs):
    import ml_dtypes
    from concourse.bass_utils import run_bass_kernel_spmd

    bf16 = ml_dtypes.bfloat16
    f = lambda k: np.asarray(inputs[k], dtype=np.float32)
    x = f("hidden_states")                        # [B, S, H]
    mask = np.asarray(inputs["attention_mask"])   # [B, S] int32
    ln1_g, ln1_b = f("ln1_g").astype(np.float64), f("ln1_b").astype(np.float64)
    ln2_g, ln2_b = f("ln2_g").astype(np.float64), f("ln2_b").astype(np.float64)
    Wq, Wk, Wv, Wo = (f(k).astype(np.float64) for k in ("Wq", "Wk", "Wv", "Wo"))
    bq, bk, bv, bo = (f(k).astype(np.float64) for k in ("bq", "bk", "bv", "bo"))
    level_logits = f("level_logits").astype(np.float64)
    Wr, br = f("Wr").astype(np.float64), f("br").astype(np.float64)
    W1, b1 = f("W1").astype(np.float64), f("b1").astype(np.float64)
    W2, b2 = f("W2").astype(np.float64), f("b2").astype(np.float64)

    # ---- host folding (as in reference, with LN1 gamma/beta absorbed) ----
    scale = 1.0 / np.sqrt(DH)
    wq_eff = (ln1_g[None, :, None] * Wq) * scale               # [L,H,H]
    bq_eff = (bq + ln1_b @ Wq) * scale                         # [L,H]
    wk_eff = ln1_g[None, :, None] * Wk
    bk_eff = bk + ln1_b @ Wk
    wv_eff = ln1_g[None, :, None] * Wv
    bv_eff = bv + ln1_b @ Wv
    lw = np.exp(level_logits - level_logits.max())
    lw = lw / lw.sum()
    wo_eff = lw[:, None, None] * Wo
    boc_eff = np.einsum("l,lh->h", lw, bo) + np.einsum("lf,lfh->h", bv_eff, wo_eff)
    wo_dev = wo_eff / SV          # device ctx carries a factor of SV

    xn1 = _ln(x.astype(np.float64))                            # LN1 sans gamma/beta
    xn1_T32 = np.ascontiguousarray(
        np.swapaxes(xn1.astype(np.float32), 1, 2))             # [B,H,S]
    mbias = (1.0 - mask.astype(np.float32)) * np.float32(-1e9)
    mb_dev = mbias + np.float32(np.log(SP))                    # exp out pre-scaled by SP

    xq8 = [_q8(xn1_T32[b], SXA) for b in range(B)]
    w8 = {}
    for l in range(L):
        for hb in range(2):
            fs = slice(hb * FB, (hb + 1) * FB)
            w8[(l, hb)] = (
                _q8(np.ascontiguousarray(wq_eff[l][:, fs].astype(np.float32)), SWQ),
                _q8(np.ascontiguousarray(wk_eff[l][:, fs].astype(np.float32)), SWK),
                _q8(np.ascontiguousarray(wv_eff[l][:, fs].astype(np.float32)), SWV),
            )
    in_maps = []
    for c in range(NCORES):
        b, g = c // 4, c % 4
        l, hb = g // 2, g % 2
        fs = slice(hb * FB, (hb + 1) * FB)
        (wqm, wqr), (wkm, wkr), (wvm, wvr) = w8[(l, hb)]
        in_maps.append({
            "xam": xq8[b][0], "xar": xq8[b][1],
            "wqm": wqm, "wqr": wqr,
            "wkm": wkm, "wkr": wkr,
            "wvm": wvm, "wvr": wvr,
            "wo": np.ascontiguousarray(wo_dev[l][fs, :].astype(np.float32)).astype(bf16),
            "bqc": _colt(bq_eff[l][fs]),
            "bkc": _colt(bk_eff[l][fs]),
            "mb": _colt(mb_dev[b]),
        })

    import time as _time
    nc_a = _get_attn()
    t0 = _time.time()
    res_a = run_bass_kernel_spmd(nc_a, in_maps, core_ids=list(range(NCORES)))
    _PERF["a_wall_s"] = _time.time() - t0

    # device xres = x + sum of partials + boc
    xres = x.reshape(-1, H).copy()
    for c in range(NCORES):
        b = c // 4
        xres[b * S:(b + 1) * S] += res_a.results[c]["po"].astype(np.float32)
    xres += boc_eff.astype(np.float32)[None, :]

    # ---- host: exact fp32 attention -> routing (LN2, router, top-2) ----
    t0 = _time.time()
    xn1h = (xn1 * ln1_g + ln1_b).astype(np.float32).reshape(-1, H)
    attn_h = np.zeros((B * S, H), np.float32)
    lw32 = lw.astype(np.float32)
    for l in range(L):
        q = (xn1h @ Wq[l].astype(np.float32) + bq[l].astype(np.float32)) \
            .reshape(B, S, NH, DH) * np.float32(scale)
        k = (xn1h @ Wk[l].astype(np.float32) + bk[l].astype(np.float32)) \
            .reshape(B, S, NH, DH)
        v = (xn1h @ Wv[l].astype(np.float32) + bv[l].astype(np.float32)) \
            .reshape(B, S, NH, DH)
        ol = np.empty((B, S, H), np.float32)
        for b_ in range(B):
            mrow = mbias[b_][None, :]
            for n in range(NH):
                sc = q[b_, :, n] @ k[b_, :, n].T + mrow
                sc -= sc.max(-1, keepdims=True)
                e = np.exp(sc)
                a = e / e.sum(-1, keepdims=True)
                ol[b_, :, n * DH:(n + 1) * DH] = a @ v[b_, :, n]
        attn_h += lw32[l] * (
            ol.reshape(-1, H) @ Wo[l].astype(np.float32) + bo[l].astype(np.float32))
    xres_h = x.reshape(-1, H) + attn_h
    xn2 = (_ln(xres_h.astype(np.float64)) * ln2_g + ln2_b)     # [T,H] fp64
    logits = xn2 @ Wr + br
    pm = logits.max(-1, keepdims=True)
    probs = np.exp(logits - pm)
    probs /= probs.sum(-1, keepdims=True)
    order = np.argsort(-probs, axis=-1, kind="stable")
    topi = order[:, :TOPK]
    topv = np.take_along_axis(probs, topi, axis=-1)
    gates = topv / topv.sum(-1, keepdims=True)                 # [T,2]
    _PERF["host_route_s"] = _time.time() - t0

    tok_idx, gate_val = [], []
    for e in range(E):
        sel = np.nonzero(topi == e)
        tok_idx.append(sel[0])
        gate_val.append(gates[sel[0], sel[1]])
    counts = [len(t) for t in tok_idx]
    C = max(512, ((max(counts) + 31) // 32) * 32)
    while True:
        nch = (C + 511) // 512
        if C % nch == 0 and C // nch >= 256:
            break
        C += 32

    w1f = ln2_g[None, :, None] * W1                            # [E,H,F]
    b1f = b1 + ln2_b @ W1                                      # [E,F]
    xn2_32 = xn2.astype(np.float32)
    e4 = ml_dtypes.float8_e4m3

    in_maps_b = []
    for e in range(E):
        xe = np.zeros((C, H), np.float32)
        xe[:counts[e]] = xn2_32[tok_idx[e]]
        xmq, xrq = _q8(np.ascontiguousarray(xe.T), SX)         # [H,C] fp8
        w1mq, w1rq = _q8(w1f[e], SW)
        w2mq, w2rq = _q8(W2[e], SW)
        g = np.zeros((1, C), np.float32)
        g[0, :counts[e]] = gate_val[e].astype(np.float32) / np.float32(SW)
        in_maps_b.append({
            "xm": xmq, "xr": xrq,
            "w1m": np.ascontiguousarray(w1mq.astype(e4)),
            "w1r": np.ascontiguousarray(w1rq.astype(e4)),
            "w2m": np.ascontiguousarray(w2mq.astype(e4)),
            "w2r": np.ascontiguousarray(w2rq.astype(e4)),
            "b1c": _colt(b1f[e]),
            "gsr": g,
        })

    nc_b = _get_expert(C)
    t0 = _time.time()
    res_b = run_bass_kernel_spmd(nc_b, in_maps_b, core_ids=list(range(NCORES)))
    _PERF["b_wall_s"] = _time.time() - t0
    _PERF["capacity"] = C
    _PERF["counts"] = counts

    out = xres
    for e in range(E):
        if counts[e]:
            out[tok_idx[e]] += res_b.results[e]["y"][:, :counts[e]].T
    # b2 contribution: sum_e gate_e * b2[e]
    gmat = np.zeros((B * S, E), np.float64)
    np.put_along_axis(gmat, topi, gates, axis=-1)
    out += (gmat @ b2).astype(np.float32)
    return out.reshape(B, S, H).astype(np.float32)


# revision 48
# speedup vs baseline: 1.2764x; 1.0380x over previous
"""Trainium2 Bass kernel for nn_MoEMLABlock (MoE + multi-level attention block).

Strategy (8 NeuronCores, full inputs in / full output out):
  Launch A (head-parallel attention): 64 attention instances (level l,
    batch b, head h) are split as: core c -> batch b=c//4, level l=(c%4)//2,
    head block hb=(c%4)%2 (8 heads / 512 feature cols).  Each projection is
    computed exactly once across cores (no K/V recompute).  LayerNorm 1,
    the 1/sqrt(DH) scale and the level-softmax weight are folded into the
    projection weights on the host; V carries no bias (its effect, plus bo,
    is the constant `boc` added on the host).  Device math is bf16 inputs
    with fp32 PSUM accumulation.  Each core emits its partial attention
    output [S, H] token-major; host sums partials + residual.
  Host: exact fp32 attention recompute (BLAS) for ROUTING ONLY -- router
    logits, top-2 and gates are bit-robust against device rounding (the
    tightest p2/p3 logit gap in this data is 3e-5, so routing must not
    depend on device numerics).  Also LN2 + expert input prep.
  Launch B (expert-parallel FFN): core e runs expert e over its routed
    tokens with error-compensated fp8 (e4m3) DoubleRow matmuls:
    x@W1 ~= xm@W1m + xr@W1m + xm@W1r where *m/*r are fp8 main/residual
    parts (measured output error 5e-4 rel, 4x PE throughput per term).
    Gates are applied on device via the activation scale; the b2 bias is
    applied on the host (gates @ b2).
  Host: scatter-add combine + residuals.
"""

import numpy as np

H = 1024
NH = 16
DH = 64
L = 2
E = 8
FF = 4096
B = 2
S = 1024
EPS = 1e-5
P = 128
NCORES = 8
KO = H // P            # 8 contraction chunks over H
TT = S // P            # 8 token tiles
FB = 512               # feature block per core (8 heads)
MF = FF // P           # 32
TOPK = 2

SX = 16.0              # fp8 scale for expert input x
SW = 64.0              # fp8 scale for expert weights
SP = 4.0               # fp8 scale for attention exp-scores
SV = 16.0              # fp8 scale for attention V
SXA = 16.0             # fp8 scale for attention input LN1(x)
SWQ = 512.0            # fp8 scale for wq (carries the 1/sqrt(DH) fold)
SWK = 64.0             # fp8 scale for wk
SWV = 64.0             # fp8 scale for wv

_CACHE = {}
_PERF = {}


def _build_attn():
    """Launch A: one (batch, level, 8-head block) per core."""
    import concourse.bacc as bacc
    import concourse.mybir as mybir
    import concourse.tile as tile

    F32, F32R, BF16 = mybir.dt.float32, mybir.dt.float32r, mybir.dt.bfloat16
    F8 = mybir.dt.float8e4
    AF = mybir.ActivationFunctionType
    DR = mybir.MatmulPerfMode.DoubleRow

    nc = bacc.Bacc()
    xam_h = nc.dram_tensor("xam", [H, S], F8, kind="ExternalInput")     # SXA*LN1(x_b)^T main
    xar_h = nc.dram_tensor("xar", [H, S], F8, kind="ExternalInput")     # ... residual
    wqm_h = nc.dram_tensor("wqm", [H, FB], F8, kind="ExternalInput")
    wqr_h = nc.dram_tensor("wqr", [H, FB], F8, kind="ExternalInput")
    wkm_h = nc.dram_tensor("wkm", [H, FB], F8, kind="ExternalInput")
    wkr_h = nc.dram_tensor("wkr", [H, FB], F8, kind="ExternalInput")
    wvm_h = nc.dram_tensor("wvm", [H, FB], F8, kind="ExternalInput")
    wvr_h = nc.dram_tensor("wvr", [H, FB], F8, kind="ExternalInput")
    wo_h = nc.dram_tensor("wo", [FB, H], BF16, kind="ExternalInput")
    bq_h = nc.dram_tensor("bqc", [P, 4], F32, kind="ExternalInput")
    bk_h = nc.dram_tensor("bkc", [P, 4], F32, kind="ExternalInput")
    mb_h = nc.dram_tensor("mb", [P, TT], F32, kind="ExternalInput")     # key mask bias + ln(SP)
    po_h = nc.dram_tensor("po", [S, H], BF16, kind="ExternalOutput")    # partial attn out

    with tile.TileContext(nc) as tc:
        with tc.tile_pool(name="consts", bufs=1) as consts, \
             tc.tile_pool(name="big", bufs=1) as big, \
             tc.tile_pool(name="work", bufs=5) as work, \
             tc.tile_pool(name="ps_mm", bufs=2, space="PSUM") as ps_mm, \
             tc.tile_pool(name="ps_sc", bufs=2, space="PSUM") as ps_sc, \
             tc.tile_pool(name="ps_cx", bufs=2, space="PSUM") as ps_cx:

            ones_f = consts.tile([P, 1], F32)
            nc.vector.memset(ones_f[:], 1.0)
            ones_row = consts.tile([1, P], F32R)
            nc.vector.tensor_copy(ones_row[:], ones_f[:1, :].to_broadcast((1, P)))

            bq_t = consts.tile([P, 4], F32)
            nc.sync.dma_start(bq_t[:], bq_h[:])
            bk_t = consts.tile([P, 4], F32)
            nc.sync.dma_start(bk_t[:], bk_h[:])
            mb_t = consts.tile([P, TT], F32)
            nc.sync.dma_start(mb_t[:], mb_h[:])

            # Two DMA queues (SP + Act): Q/K path inputs on SP in first-use
            # order; V weights + wo in parallel on the Act queue.
            def ld(dst, src, eng):
                eng.dma_start(dst[:], src.rearrange("(ko p) f -> p ko f", p=P))

            wqm_t = big.tile([P, KO, FB], F8)
            ld(wqm_t, wqm_h[:], nc.sync)
            wqr_t = big.tile([P, KO, FB], F8)
            ld(wqr_t, wqr_h[:], nc.sync)
            xm_a = big.tile([P, KO, FB], F8)       # tokens 0..511
            ld(xm_a, xam_h[:, :FB], nc.sync)
            xr_a = big.tile([P, KO, FB], F8)
            ld(xr_a, xar_h[:, :FB], nc.sync)
            xm_b = big.tile([P, KO, FB], F8)       # tokens 512..1023
            ld(xm_b, xam_h[:, FB:], nc.sync)
            xr_b = big.tile([P, KO, FB], F8)
            ld(xr_b, xar_h[:, FB:], nc.sync)
            wkm_t = big.tile([P, KO, FB], F8)
            ld(wkm_t, wkm_h[:], nc.sync)
            wkr_t = big.tile([P, KO, FB], F8)
            ld(wkr_t, wkr_h[:], nc.sync)
            wvm_t = big.tile([P, KO, FB], F8)
            ld(wvm_t, wvm_h[:], nc.scalar)
            wvr_t = big.tile([P, KO, FB], F8)
            ld(wvr_t, wvr_h[:], nc.scalar)
            wo_t = big.tile([P, 4, H], BF16)
            nc.scalar.dma_start(wo_t[:], wo_h[:].rearrange("(fc p) h -> p fc h", p=P))

            def xtok(src, t0):
                # [P, KO, 128] token block starting at t0 (main or residual)
                return src[:, :, t0 % FB:t0 % FB + P]

            # ---- tiles; emission order below keeps the Act exp chain (the
            #      pacer) fed from early on while PE fills with proj work ----
            v_t = big.tile([P, TT, 8, DH + 2], F8)
            nc.vector.memset(v_t[:, :, :, DH:DH + 1], 1.0)
            nc.vector.memset(v_t[:, :, :, DH + 1:DH + 2], 0.0)
            q_t = big.tile([P, 4, S], BF16)
            k_t = big.tile([P, 4, S], BF16)
            ctx = big.tile([P, 4, S], BF16)
            p_sbs = {}

            def comp_mm(out_ap, x_of, w_of, x_is_lhs):
                """3-term compensated fp8 DoubleRow accumulation group:
                xm@wm + xr@wm + xm@wr."""
                terms = [(0, 0), (1, 0), (0, 1)]   # (x residual?, w residual?)
                for ti, (xi, wi) in enumerate(terms):
                    for kc2 in range(KO // 2):
                        ks = slice(2 * kc2, 2 * kc2 + 2)
                        xa, wa = x_of(xi, ks), w_of(wi, ks)
                        lhsT, rhs = (xa, wa) if x_is_lhs else (wa, xa)
                        nc.tensor.matmul(
                            out_ap, lhsT, rhs,
                            start=(ti == 0 and kc2 == 0),
                            stop=(ti == 2 and kc2 == KO // 2 - 1),
                            perf_mode=DR,
                        )

            def emit_v(half):
                for tt in range(half * 4, half * 4 + 4):
                    xmx = xm_a if tt < 4 else xm_b
                    xrx = xr_a if tt < 4 else xr_b
                    vps = ps_mm.tile([P, FB], F32, tag="mm")
                    comp_mm(
                        vps[:],
                        lambda xi, ks: xtok(xrx if xi else xmx, tt * P)[:, ks, :],
                        lambda wi, ks: (wvr_t if wi else wvm_t)[:, ks, :],
                        x_is_lhs=True,
                    )
                    nc.vector.tensor_scalar_mul(
                        v_t[:, tt, :, 0:DH],
                        vps[:].rearrange("p (h d) -> p h d", d=DH),
                        float(SV / (SXA * SWV)),
                    )

            def emit_qk(ft):
                for dst, wm, wr, bsrc, sw in (
                        (q_t, wqm_t, wqr_t, bq_t, SWQ),
                        (k_t, wkm_t, wkr_t, bk_t, SWK)):
                    for qc in range(2):
                        xmx = xm_a if qc == 0 else xm_b
                        xrx = xr_a if qc == 0 else xr_b
                        pps = ps_mm.tile([P, FB], F32, tag="mm")
                        comp_mm(
                            pps[:],
                            lambda xi, ks: (xrx if xi else xmx)[:, ks, :],
                            lambda wi, ks: (wr if wi else wm)[:, ks, ft * P:(ft + 1) * P],
                            x_is_lhs=False,
                        )
                        nc.vector.tensor_scalar(
                            dst[:, ft, qc * FB:(qc + 1) * FB], pps[:],
                            float(1.0 / (SXA * sw)), bsrc[:, ft:ft + 1],
                            op0=mybir.AluOpType.mult, op1=mybir.AluOpType.add,
                        )

            def emit_sc(h):
                ft, pb = h // 2, (h % 2) * DH
                p_sb = work.tile([P, TT, S], F8, tag="p_sb")
                p_sbs[h] = p_sb
                for tt in range(TT):
                    sps = ps_sc.tile([P, S], F32, tag="sc")
                    for qc in range(2):
                        nc.tensor.matmul(
                            sps[:, qc * FB:(qc + 1) * FB],
                            k_t[pb:pb + DH, ft, tt * P:(tt + 1) * P],
                            q_t[pb:pb + DH, ft, qc * FB:(qc + 1) * FB],
                            start=True, stop=True,
                        )
                    nc.scalar.activation(
                        p_sb[:, tt, :], sps[:],
                        AF.Exp, bias=mb_t[:, tt:tt + 1],
                    )

            def emit_ctx(h):
                ft, pb = h // 2, (h % 2) * DH
                p_sb = p_sbs[h]
                for qc in range(2):
                    cxps = ps_cx.tile([DH + 2, FB], F32, tag="cx")
                    for t2 in range(TT // 2):
                        nc.tensor.matmul(
                            cxps[:],
                            v_t[:, 2 * t2:2 * t2 + 2, h, :],
                            p_sb[:, 2 * t2:2 * t2 + 2, qc * FB:(qc + 1) * FB],
                            start=(t2 == 0), stop=(t2 == TT // 2 - 1),
                            perf_mode=DR,
                        )
                    r = work.tile([1, FB], F32R, tag="r")
                    with nc.allow_low_precision(reason="softmax recip feeds broadcast matmul"):
                        nc.vector.reciprocal(r[:], cxps[DH:DH + 1, :])
                    rbps = ps_mm.tile([P, FB], F32, tag="mm")
                    nc.tensor.matmul(rbps[:DH, :], ones_row[:, :DH], r[:], start=True, stop=True)
                    rb_sb = work.tile([DH, FB], F32, tag="rb_sb")
                    nc.vector.tensor_copy(rb_sb[:], rbps[:DH, :])
                    nc.vector.tensor_mul(
                        ctx[pb:pb + DH, ft, qc * FB:(qc + 1) * FB],
                        cxps[0:DH, :], rb_sb[:],
                    )

            emit_qk(0)
            emit_sc(0)
            emit_sc(1)
            emit_v(0)
            emit_qk(1)
            emit_sc(2)
            emit_v(1)
            emit_ctx(0)
            emit_sc(3)
            emit_ctx(1)
            emit_qk(2)
            emit_sc(4)
            emit_sc(5)
            emit_ctx(2)
            emit_ctx(3)
            emit_qk(3)
            emit_sc(6)
            emit_sc(7)
            emit_ctx(4)
            emit_ctx(5)
            emit_ctx(6)
            emit_ctx(7)

            # ---- O-projection straight to token-major, DMA out ----
            po_t = big.tile([P, TT, H], BF16)
            for tt in range(TT):
                for hc in range(2):
                    ops = ps_mm.tile([P, FB], F32, tag="mm")
                    for fc in range(4):
                        nc.tensor.matmul(
                            ops[:],
                            ctx[:, fc, tt * P:(tt + 1) * P],
                            wo_t[:, fc, hc * FB:(hc + 1) * FB],
                            start=(fc == 0), stop=(fc == 3),
                        )
                    nc.vector.tensor_copy(po_t[:, tt, hc * FB:(hc + 1) * FB], ops[:])
                nc.sync.dma_start(
                    po_h[:].rearrange("(tt p) h -> p tt h", p=P)[:, tt, :],
                    po_t[:, tt, :],
                )

    nc.finalize()
    return nc


def _build_expert(C):
    """Launch B: one expert FFN over C routed tokens, compensated fp8.

    Feature-major throughout: both passes cost ~C columns (no 128-token
    tile rounding), so C only needs 32-alignment.  Output y is [H, C];
    the host transposes.  Gates (pre-divided by SW) are broadcast to all
    partitions via a ones-matmul and applied on the DVE.
    """
    import concourse.bacc as bacc
    import concourse.mybir as mybir
    import concourse.tile as tile

    F32, F32R, F8 = mybir.dt.float32, mybir.dt.float32r, mybir.dt.float8e4
    AF = mybir.ActivationFunctionType
    DR = mybir.MatmulPerfMode.DoubleRow

    assert C % 32 == 0 and C >= 512, C
    NCH = max(1, (C + 511) // 512)
    assert C % NCH == 0, (C, NCH)
    CN = C // NCH
    assert 256 <= CN <= 512, CN

    nc = bacc.Bacc()
    xm_h = nc.dram_tensor("xm", [H, C], F8, kind="ExternalInput")   # SX*x main
    xr_h = nc.dram_tensor("xr", [H, C], F8, kind="ExternalInput")   # SX*x residual
    w1m_h = nc.dram_tensor("w1m", [H, FF], F8, kind="ExternalInput")
    w1r_h = nc.dram_tensor("w1r", [H, FF], F8, kind="ExternalInput")
    w2m_h = nc.dram_tensor("w2m", [FF, H], F8, kind="ExternalInput")
    w2r_h = nc.dram_tensor("w2r", [FF, H], F8, kind="ExternalInput")
    b1_h = nc.dram_tensor("b1c", [P, MF], F32, kind="ExternalInput")
    g_h = nc.dram_tensor("gsr", [1, C], F32, kind="ExternalInput")  # gate/SW per token
    y_h = nc.dram_tensor("y", [H, C], F32, kind="ExternalOutput")   # feature-major

    with tile.TileContext(nc) as tc:
        with tc.tile_pool(name="consts", bufs=1) as consts, \
             tc.tile_pool(name="big", bufs=1) as big, \
             tc.tile_pool(name="w1s", bufs=3) as w1s, \
             tc.tile_pool(name="ht", bufs=2) as htp, \
             tc.tile_pool(name="ps_mm", bufs=3, space="PSUM") as ps_mm:

            b1_t = consts.tile([P, MF], F32)
            nc.sync.dma_start(b1_t[:], b1_h[:])
            ones_f = consts.tile([P, 1], F32)
            nc.vector.memset(ones_f[:], 1.0)
            ones_row = consts.tile([1, P], F32R)
            nc.vector.tensor_copy(ones_row[:], ones_f[:1, :].to_broadcast((1, P)))
            g_sb = consts.tile([1, C], F32R)
            nc.sync.dma_start(g_sb[:], g_h[:].bitcast(F32R))

            # x on the Act DMA queue (idle early); W2 interleaved into the
            # SP W1-chunk stream (slack-scheduled)
            xm_t = big.tile([P, KO, C], F8)
            nc.scalar.dma_start(xm_t[:], xm_h[:].rearrange("(ko p) c -> p ko c", p=P))
            xr_t = big.tile([P, KO, C], F8)
            nc.scalar.dma_start(xr_t[:], xr_h[:].rearrange("(ko p) c -> p ko c", p=P))
            w2m_t = big.tile([P, MF, H], F8)
            w2r_t = big.tile([P, MF, H], F8)

            hm_t = big.tile([P, MF, C], F8)
            hr_t = big.tile([P, MF, C], F8)

            # gate row broadcast to all partitions (gb[p, c] = gate_c / SW)
            gb_sb = big.tile([P, C], F32)
            for nch in range(NCH):
                cs = slice(nch * CN, (nch + 1) * CN)
                gps = ps_mm.tile([P, 512], F32, tag="mm")
                nc.tensor.matmul(gps[:, :CN], ones_row[:], g_sb[:, cs],
                                 start=True, stop=True)
                nc.vector.tensor_copy(gb_sb[:, cs], gps[:, :CN])

            # ---- W1 pass: h = gelu((xm+xr)@(w1m+w1r)/(SX*SW) + b1) ----
            for mfc in range(FF // FB):            # 8 chunks of 512 cols
                w1m_c = w1s.tile([P, KO, FB], F8, tag="w1m")
                nc.sync.dma_start(
                    w1m_c[:],
                    w1m_h[:, mfc * FB:(mfc + 1) * FB].rearrange("(ko p) f -> p ko f", p=P),
                )
                w1r_c = w1s.tile([P, KO, FB], F8, tag="w1r")
                nc.sync.dma_start(
                    w1r_c[:],
                    w1r_h[:, mfc * FB:(mfc + 1) * FB].rearrange("(ko p) f -> p ko f", p=P),
                )
                # one 1MB W2 piece per chunk boundary (slack-scheduled)
                pi = mfc
                wt = w2m_t if pi < 4 else w2r_t
                wh = w2m_h if pi < 4 else w2r_h
                qi = pi % 4
                nc.sync.dma_start(
                    wt[:, 8 * qi:8 * qi + 8, :],
                    wh[qi * 1024:(qi + 1) * 1024, :].rearrange(
                        "(fc p) h -> p fc h", p=P),
                )
                for mf in range(4):
                    mfg = mfc * 4 + mf
                    for nch in range(NCH):
                        cs = slice(nch * CN, (nch + 1) * CN)
                        hps = ps_mm.tile([P, 512], F32, tag="mm")
                        terms = [(w1m_c, xm_t), (w1m_c, xr_t), (w1r_c, xm_t)]
                        nterm = len(terms)
                        for ti, (wt, xt) in enumerate(terms):
                            for kc2 in range(KO // 2):
                                nc.tensor.matmul(
                                    hps[:, :CN],
                                    wt[:, 2 * kc2:2 * kc2 + 2, mf * P:(mf + 1) * P],
                                    xt[:, 2 * kc2:2 * kc2 + 2, cs],
                                    start=(ti == 0 and kc2 == 0),
                                    stop=(ti == nterm - 1 and kc2 == KO // 2 - 1),
                                    perf_mode=DR,
                                )
                        ht = htp.tile([P, 512], F32, tag="ht")
                        nc.scalar.activation(
                            ht[:, :CN], hps[:, :CN], AF.Gelu_apprx_tanh,
                            bias=b1_t[:, mfg:mfg + 1], scale=1.0 / (SX * SW),
                        )
                        # split the fp8 quantize-copy across Act/DVE
                        if mfg % 2 == 0:
                            nc.scalar.activation(hm_t[:, mfg, cs], ht[:, :CN], AF.Copy)
                        else:
                            nc.vector.tensor_copy(hm_t[:, mfg, cs], ht[:, :CN])
                        nc.vector.tensor_sub(hr_t[:, mfg, cs], ht[:, :CN], hm_t[:, mfg, cs])

            # ---- W2 pass, feature-major: y[h, c] = (h@W2)[h, c] * gb[c] ----
            y_t = big.tile([P, KO, C], F32)
            for hc in range(KO):
                for nch in range(NCH):
                    cs = slice(nch * CN, (nch + 1) * CN)
                    yps = ps_mm.tile([P, 512], F32, tag="mm")
                    terms = [(hm_t, w2m_t), (hr_t, w2m_t), (hm_t, w2r_t)]
                    nterm = len(terms)
                    for ti, (ht_, wt) in enumerate(terms):
                        for fc2 in range(MF // 2):
                            nc.tensor.matmul(
                                yps[:, :CN],
                                wt[:, 2 * fc2:2 * fc2 + 2, hc * P:(hc + 1) * P],
                                ht_[:, 2 * fc2:2 * fc2 + 2, cs],
                                start=(ti == 0 and fc2 == 0),
                                stop=(ti == nterm - 1 and fc2 == MF // 2 - 1),
                                perf_mode=DR,
                            )
                    nc.vector.tensor_mul(
                        y_t[:, hc, cs], yps[:, :CN], gb_sb[:, cs],
                    )
                nc.sync.dma_start(
                    y_h[:].rearrange("(hc p) c -> p hc c", p=P)[:, hc, :],
                    y_t[:, hc, :],
                )

    nc.finalize()
    return nc


def _get_attn():
    if "attn" not in _CACHE:
        _CACHE["attn"] = _build_attn()
    return _CACHE["attn"]


def _get_expert(C):
    key = ("exp", C)
    if key not in _CACHE:
        _CACHE[key] = _build_expert(C)
    return _CACHE[key]


def _ln(x):
    m = x.mean(-1, keepdims=True)
    v = x.var(-1, keepdims=True)
    return (x - m) / np.sqrt(v + EPS)


def _q8(a, s):
    """e4m3 quantize a*s, return (main, residual) as raw scaled fp8 arrays."""
    import ml_dtypes
    e4 = ml_dtypes.float8_e4m3
    m = (a * s).astype(np.float32).astype(e4)
    r = ((a * s).astype(np.float32) - m.astype(np.float32)).astype(e4)
    return m, r


def _colt(vec):
    v32 = np.ascontiguousarray(np.asarray(vec, dtype=np.float32))
    return np.ascontiguousarray(v32.reshape(-1, P).T)


def kernel(**inputs):
    import ml_dtypes
    from concourse.bass_utils import run_bass_kernel_spmd

    bf16 = ml_dtypes.bfloat16
    f = lambda k: np.asarray(inputs[k], dtype=np.float32)
    x = f("hidden_states")                        # [B, S, H]
    mask = np.asarray(inputs["attention_mask"])   # [B, S] int32
    ln1_g, ln1_b = f("ln1_g").astype(np.float64), f("ln1_b").astype(np.float64)
    ln2_g, ln2_b = f("ln2_g").astype(np.float64), f("ln2_b").astype(np.float64)
    Wq, Wk, Wv, Wo = (f(k).astype(np.float64) for k in ("Wq", "Wk", "Wv", "Wo"))
    bq, bk, bv, bo = (f(k).astype(np.float64) for k in ("bq", "bk", "bv", "bo"))
    level_logits = f("level_logits").astype(np.float64)
    Wr, br = f("Wr").astype(np.float64), f("br").astype(np.float64)
    W1, b1 = f("W1").astype(np.float64), f("b1").astype(np.float64)
    W2, b2 = f("W2").astype(np.float64), f("b2").astype(np.float64)

    # ---- host folding (as in reference, with LN1 gamma/beta absorbed) ----
    scale = 1.0 / np.sqrt(DH)
    wq_eff = (ln1_g[None, :, None] * Wq) * scale               # [L,H,H]
    bq_eff = (bq + ln1_b @ Wq) * scale                         # [L,H]
    wk_eff = ln1_g[None, :, None] * Wk
    bk_eff = bk + ln1_b @ Wk
    wv_eff = ln1_g[None, :, None] * Wv
    bv_eff = bv + ln1_b @ Wv
    lw = np.exp(level_logits - level_logits.max())
    lw = lw / lw.sum()
    wo_eff = lw[:, None, None] * Wo
    boc_eff = np.einsum("l,lh->h", lw, bo) + np.einsum("lf,lfh->h", bv_eff, wo_eff)
    wo_dev = wo_eff / SV          # device ctx carries a factor of SV

    xn1 = _ln(x.astype(np.float64))                            # LN1 sans gamma/beta
    xn1_T32 = np.ascontiguousarray(
        np.swapaxes(xn1.astype(np.float32), 1, 2))             # [B,H,S]
    mbias = (1.0 - mask.astype(np.float32)) * np.float32(-1e9)
    mb_dev = mbias + np.float32(np.log(SP))                    # exp out pre-scaled by SP

    xq8 = [_q8(xn1_T32[b], SXA) for b in range(B)]
    w8 = {}
    for l in range(L):
        for hb in range(2):
            fs = slice(hb * FB, (hb + 1) * FB)
            w8[(l, hb)] = (
                _q8(np.ascontiguousarray(wq_eff[l][:, fs].astype(np.float32)), SWQ),
                _q8(np.ascontiguousarray(wk_eff[l][:, fs].astype(np.float32)), SWK),
                _q8(np.ascontiguousarray(wv_eff[l][:, fs].astype(np.float32)), SWV),
            )
    in_maps = []
    for c in range(NCORES):
        b, g = c // 4, c % 4
        l, hb = g // 2, g % 2
        fs = slice(hb * FB, (hb + 1) * FB)
        (wqm, wqr), (wkm, wkr), (wvm, wvr) = w8[(l, hb)]
        in_maps.append({
            "xam": xq8[b][0], "xar": xq8[b][1],
            "wqm": wqm, "wqr": wqr,
            "wkm": wkm, "wkr": wkr,
            "wvm": wvm, "wvr": wvr,
            "wo": np.ascontiguousarray(wo_dev[l][fs, :].astype(np.float32)).astype(bf16),
            "bqc": _colt(bq_eff[l][fs]),
            "bkc": _colt(bk_eff[l][fs]),
            "mb": _colt(mb_dev[b]),
        })

    import time as _time
    nc_a = _get_attn()
    t0 = _time.time()
    res_a = run_bass_kernel_spmd(nc_a, in_maps, core_ids=list(range(NCORES)))
    _PERF["a_wall_s"] = _time.time() - t0

    # device xres = x + sum of partials + boc
    xres = x.reshape(-1, H).copy()
    for c in range(NCORES):
        b = c // 4
        xres[b * S:(b + 1) * S] += res_a.results[c]["po"].astype(np.float32)
    xres += boc_eff.astype(np.float32)[None, :]

    # ---- host: exact fp32 attention -> routing (LN2, router, top-2) ----
    t0 = _time.time()
    xn1h = (xn1 * ln1_g + ln1_b).astype(np.float32).reshape(-1, H)
    attn_h = np.zeros((B * S, H), np.float32)
    lw32 = lw.astype(np.float32)
    for l in range(L):
        q = (xn1h @ Wq[l].astype(np.float32) + bq[l].astype(np.float32)) \
            .reshape(B, S, NH, DH) * np.float32(scale)
        k = (xn1h @ Wk[l].astype(np.float32) + bk[l].astype(np.float32)) \
            .reshape(B, S, NH, DH)
        v = (xn1h @ Wv[l].astype(np.float32) + bv[l].astype(np.float32)) \
            .reshape(B, S, NH, DH)
        ol = np.empty((B, S, H), np.float32)
        for b_ in range(B):
            mrow = mbias[b_][None, :]
            for n in range(NH):
                sc = q[b_, :, n] @ k[b_, :, n].T + mrow
                sc -= sc.max(-1, keepdims=True)
                e = np.exp(sc)
                a = e / e.sum(-1, keepdims=True)
                ol[b_, :, n * DH:(n + 1) * DH] = a @ v[b_, :, n]
        attn_h += lw32[l] * (
            ol.reshape(-1, H) @ Wo[l].astype(np.float32) + bo[l].astype(np.float32))
    xres_h = x.reshape(-1, H) + attn_h
    xn2 = (_ln(xres_h.astype(np.float64)) * ln2_g + ln2_b)     # [T,H] fp64
    logits = xn2 @ Wr + br
    pm = logits.max(-1, keepdims=True)
    probs = np.exp(logits - pm)
    probs /= probs.sum(-1, keepdims=True)
    order = np.argsort(-probs, axis=-1, kind="stable")
    topi = order[:, :TOPK]
    topv = np.take_along_axis(probs, topi, axis=-1)
    gates = topv / topv.sum(-1, keepdims=True)                 # [T,2]
    _PERF["host_route_s"] = _time.time() - t0

    tok_idx, gate_val = [], []
    for e in range(E):
        sel = np.nonzero(topi == e)
        tok_idx.append(sel[0])
        gate_val.append(gates[sel[0], sel[1]])
    counts = [len(t) for t in tok_idx]
    C = max(512, ((max(counts) + 31) // 32) * 32)
    while True:
        nch = (C + 511) // 512
        if C % nch == 0 and C // nch >= 256:
            break
        C += 32

    w1f = ln2_g[None, :, None] * W1                            # [E,H,F]
    b1f = b1 + ln2_b @ W1                                      # [E,F]
    xn2_32 = xn2.astype(np.float32)
    e4 = ml_dtypes.float8_e4m3

    in_maps_b = []
    for e in range(E):
        xe = np.zeros((C, H), np.float32)
        xe[:counts[e]] = xn2_32[tok_idx[e]]
        xmq, xrq = _q8(np.ascontiguousarray(xe.T), SX)         # [H,C] fp8
        w1mq, w1rq = _q8(w1f[e], SW)
        w2mq, w2rq = _q8(W2[e], SW)
        g = np.zeros((1, C), np.float32)
        g[0, :counts[e]] = gate_val[e].astype(np.float32) / np.float32(SW)
        in_maps_b.append({
            "xm": xmq, "xr": xrq,
            "w1m": np.ascontiguousarray(w1mq.astype(e4)),
            "w1r": np.ascontiguousarray(w1rq.astype(e4)),
            "w2m": np.ascontiguousarray(w2mq.astype(e4)),
            "w2r": np.ascontiguousarray(w2rq.astype(e4)),
            "b1c": _colt(b1f[e]),
            "gsr": g,
        })

    nc_b = _get_expert(C)
    t0 = _time.time()
    res_b = run_bass_kernel_spmd(nc_b, in_maps_b, core_ids=list(range(NCORES)))
    _PERF["b_wall_s"] = _time.time() - t0
    _PERF["capacity"] = C
    _PERF["counts"] = counts

    out = xres
    for e in range(E):
        if counts[e]:
            out[tok_idx[e]] += res_b.results[e]["y"][:, :counts[e]].T
    # b2 contribution: sum_e gate_e * b2[e]
    gmat = np.zeros((B * S, E), np.float64)
    np.put_along_axis(gmat, topi, gates, axis=-1)
    out += (gmat @ b2).astype(np.float32)
    return out.reshape(B, S, H).astype(np.float32)
